# revision 1
# baseline (speedup 1.0000x reference)
"""Trainium2 Bass kernel for a GQA attention block (B=1, T=2048, C=4096,
NH=32, NKV=8, HS=128), tensor-parallel over heads across 8 NeuronCores.

Per core c: 4 query heads (4c..4c+3) and 1 KV head (c).
  - projections computed in natural layout (lhsT = x^T tile stationary)
  - RoPE applied on natural q/k tiles (free-dim rotate-half)
  - q,k transposed via PE into [HS, T] layout for attention
  - scores computed transposed (S^T [keys, queries]); softmax denominator
    accumulated via an extra ones-matmul; causal mask via 0/1 mask multiply
  - y^T accumulated in PSUM, normalized by 1/l, c_proj partial computed
    against Wc column-slice; partials summed on host (the TP all-reduce).

All heavy matmuls use float32r (full PE speed at N>=256, ~1e-4 rel err).
"""
import sys
import os

sys.path.insert(0, "/opt/trn_rl_repo")

import numpy as np

from contextlib import ExitStack

import concourse.bass as bass
import concourse.mybir as mybir
import concourse.tile as tile
from concourse.bass_utils import run_bass_kernel_spmd

# ---------------------------------------------------------------- constants
B, T, C = 1, 2048, 4096
NH, NKV, HS = 32, 8, 128
NCORES = 8
QH = NH // NCORES          # 4 query heads per core
DQ = QH * HS               # 512
NTM = T // 128             # 16 T-chunks
NKC = C // 128             # 32 contraction chunks
NQB = T // 512             # 4 query blocks
BASE, SCALE = 10000.0, 1.0
INV_SQRT_HS = 1.0 / float(np.sqrt(HS))

F32 = mybir.dt.float32
F32R = mybir.dt.float32r

# ------------------------------------------------------- wait legalization
_TAIL_RUNWAY = 48


def _legalize_waits(nc):
    """walrus (this toolchain) allows ONE sync wait per ISA instruction.
    Split excess waits off onto standalone EventSemaphore instructions
    inserted immediately before the offender (same engine stream order)."""
    n_split = 0
    for bb in nc.m.functions[0].blocks:
        insts = bb.instructions
        if not any(i.sync_info and i.sync_info.on_wait and
                   len(i.sync_info.on_wait) > (0 if type(i).__name__ == "InstISA" else 1)
                   for i in insts):
            continue
        new_list = []
        for inst in insts:
            si = inst.sync_info
            is_raw_isa = type(inst).__name__ == "InstISA"
            keep_n = 0 if is_raw_isa else 1
            if si and si.on_wait and len(si.on_wait) > keep_n:
                waits = list(si.on_wait)
                split_off = waits if is_raw_isa else waits[:-1]
                for w in split_off:
                    ev = mybir.InstNoOp(
                        name=f"legal-wait-{nc.next_id()}",
                        ins=[], outs=[], engine=inst.engine,
                        bass_nofuse=True,
                        sync_info=mybir.SyncInfo(on_wait=[w], on_update=[]))
                    nc.register_instruction(ev, overwrite=True)
                    new_list.append(ev)
                    n_split += 1
                inst.sync_info = mybir.SyncInfo(
                    on_wait=[] if is_raw_isa else [waits[-1]],
                    on_update=list(si.on_update))
            new_list.append(inst)
        bb.instructions = new_list
    return n_split


def _audit(nc):
    bad = []
    for bb in nc.m.functions[0].blocks:
        for inst in bb.instructions:
            si = inst.sync_info
            if si and si.on_wait and len(si.on_wait) > 1:
                bad.append((type(inst).__name__, inst.name, str(inst.engine),
                            len(si.on_wait)))
    return bad


class _TailRunwayPatch:
    """Plant runway nops on SP right before Tile's tail drain so the drain's
    many queue waits can be redistributed by _legalize_waits."""

    def __enter__(self):
        self.orig = tile.TileContext._drain_and_barrier
        orig = self.orig

        def patched(tc_self, tick_clock, wait_clock):
            for _ in range(_TAIL_RUNWAY):
                tc_self.nc.sync.nop(nofuse=True)
            return orig(tc_self, tick_clock, wait_clock)

        tile.TileContext._drain_and_barrier = patched
        return self

    def __exit__(self, *a):
        tile.TileContext._drain_and_barrier = self.orig


# ---------------------------------------------------------------- builder

def _build_nc():
    nc = bass.Bass(trn_type="TRN2")

    xt = nc.dram_tensor("xt", [C, T], F32R, kind="ExternalInput")
    wqkv = nc.dram_tensor("wqkv", [C, DQ + 2 * HS], F32R, kind="ExternalInput")
    wc = nc.dram_tensor("wc", [DQ, C], F32R, kind="ExternalInput")
    cs = nc.dram_tensor("cs", [T, HS], F32, kind="ExternalInput")
    sn = nc.dram_tensor("sn", [T, HS // 2], F32, kind="ExternalInput")
    masks = nc.dram_tensor("masks", [4 * 128, 512], F32R, kind="ExternalInput")
    ones_t = nc.dram_tensor("ones_t", [128, 128], F32R, kind="ExternalInput")
    bqbc = nc.dram_tensor("bqbc", [128, DQ], F32, kind="ExternalInput")
    bvcol = nc.dram_tensor("bvcol", [128, 1], F32, kind="ExternalInput")
    ident = nc.dram_tensor("ident", [128, 128], F32, kind="ExternalInput")
    out = nc.dram_tensor("out", [T, C], F32, kind="ExternalOutput")

    with _TailRunwayPatch(), tile.TileContext(nc) as tc:
        _trace_body(nc, tc, xt, wqkv, wc, cs, sn, masks, ones_t, bqbc, bvcol,
                    ident, out)

    _legalize_waits(nc)
    bad = _audit(nc)
    if bad:
        raise RuntimeError(f"multi-wait instructions remain: {bad[:10]}")
    return nc


def _dummy_mm(nc, ps_ap, ones_bf):
    """Tiny bf16 matmul into ps_ap[0:1,0:2] to absorb the PSUM WAR wait."""
    nc.tensor.matmul(ps_ap[0:1, 0:2], ones_bf[:, 0:1], ones_bf[:, 0:2],
                     start=True, stop=True, skip_group_check=True)


def _trace_body(nc, tc, xt, wqkv, wc, cs, sn, masks, ones_t, bqbc, bvcol,
                ident, out):
    persist = ExitStack()

    # ---------------- persistent pools (whole kernel) ----------------
    misc = persist.enter_context(tc.tile_pool(name="misc", bufs=1))
    v_pool = persist.enter_context(tc.tile_pool(name="vsb", bufs=1))
    qkt_pool = persist.enter_context(tc.tile_pool(name="qkt", bufs=1))

    ones_full = misc.tile([128, 128], F32R)
    nc.sync.dma_start(out=ones_full, in_=ones_t[:, :])
    ones_sb = ones_full
    ones_bf = misc.tile([128, 2], mybir.dt.bfloat16)
    nc.vector.tensor_copy(out=ones_bf, in_=ones_full[:, 0:2])
    mask_sb = misc.tile([128, 4, 512], F32R)
    for o in range(4):
        nc.sync.dma_start(out=mask_sb[:, o, :], in_=masks[o * 128:(o + 1) * 128, :])
    bq_sb = misc.tile([128, DQ], F32)
    nc.sync.dma_start(out=bq_sb, in_=bqbc[:, :])
    bv_sb = misc.tile([128, 1], F32)
    nc.sync.dma_start(out=bv_sb, in_=bvcol[:, :])
    ident_sb = misc.tile([128, 128], F32)
    nc.sync.dma_start(out=ident_sb, in_=ident[:, :])

    v_sb = v_pool.tile([128, NTM, HS], F32R)          # V natural [T, HS]
    qkT = qkt_pool.tile([128, QH + 1, T], F32R)       # q heads 0..3, k at 4

    # ---------------- phase 1+2: projections, RoPE, transpose --------
    ph12 = ExitStack()
    w_pool = ph12.enter_context(tc.tile_pool(name="wqkv", bufs=1))
    wqkv_sb = w_pool.tile([128, NKC, DQ + 2 * HS], F32R)
    for kc in range(NKC):
        nc.sync.dma_start(out=wqkv_sb[:, kc, :], in_=wqkv[kc * 128:(kc + 1) * 128, :])

    xt_pool = ph12.enter_context(tc.tile_pool(name="xt", bufs=2))
    qn_pool = ph12.enter_context(tc.tile_pool(name="qnat", bufs=3))
    kn_pool = ph12.enter_context(tc.tile_pool(name="knat", bufs=3))
    cs_pool = ph12.enter_context(tc.tile_pool(name="cossin", bufs=2))
    t1_pool = ph12.enter_context(tc.tile_pool(name="ropetmp", bufs=3))
    ps12 = ph12.enter_context(tc.tile_pool(name="ps12", bufs=1, space="PSUM"))
    psq = ph12.enter_context(tc.tile_pool(name="psq", bufs=2, space="PSUM"))
    pskv = ph12.enter_context(tc.tile_pool(name="pskv", bufs=2, space="PSUM"))
    pstr = ph12.enter_context(tc.tile_pool(name="pstr", bufs=2, space="PSUM"))


    for tm in range(NTM):
        xt_sb = xt_pool.tile([128, NKC, 128], F32R)
        for kc in range(NKC):
            nc.sync.dma_start(out=xt_sb[:, kc, :],
                              in_=xt[kc * 128:(kc + 1) * 128,
                                     tm * 128:(tm + 1) * 128])
        q_ps = psq.tile([128, DQ], F32)
        kv_ps = pskv.tile([128, 2 * HS], F32)
        for kc in range(NKC):
            nc.tensor.matmul(q_ps, xt_sb[:, kc, :], wqkv_sb[:, kc, 0:DQ],
                             start=(kc == 0), stop=(kc == NKC - 1),
                             skip_group_check=True)
            nc.tensor.matmul(kv_ps, xt_sb[:, kc, :],
                             wqkv_sb[:, kc, DQ:DQ + 2 * HS],
                             start=(kc == 0), stop=(kc == NKC - 1),
                             skip_group_check=True)
        # drains (natural layout, fp32)
        q_nat = qn_pool.tile([128, DQ], F32)
        nc.scalar.copy(out=q_nat, in_=q_ps)
        k_nat = kn_pool.tile([128, HS], F32)
        nc.scalar.copy(out=k_nat, in_=kv_ps[:, 0:HS])
        nc.scalar.copy(out=v_sb[:, tm, :], in_=kv_ps[:, HS:2 * HS])

        # bq (pre-RoPE, exact)
        nc.vector.tensor_add(q_nat, q_nat, bq_sb)

        # RoPE + transpose per head surface (0..3 = q heads, 4 = k)
        cs_sb = cs_pool.tile([128, HS], F32)
        nc.sync.dma_start(out=cs_sb, in_=cs[tm * 128:(tm + 1) * 128, :])
        sn_sb = cs_pool.tile([128, HS // 2], F32)
        nc.sync.dma_start(out=sn_sb, in_=sn[tm * 128:(tm + 1) * 128, :])
        for s in range(QH + 1):
            src = q_nat[:, s * HS:(s + 1) * HS] if s < QH else k_nat[:, :]
            t1 = t1_pool.tile([128, HS], F32)
            nc.vector.tensor_mul(t1[:, 0:64], src[:, 64:128], sn_sb)
            nc.vector.tensor_mul(t1[:, 64:128], src[:, 0:64], sn_sb)
            nc.vector.tensor_mul(src, src, cs_sb)
            nc.vector.tensor_sub(src[:, 0:64], src[:, 0:64], t1[:, 0:64])
            nc.vector.tensor_add(src[:, 64:128], src[:, 64:128], t1[:, 64:128])
            tr_ps = pstr.tile([128, 128], F32)
            nc.tensor.matmul(tr_ps, src, ident_sb, is_transpose=True,
                             skip_group_check=True)
            nc.scalar.copy(out=qkT[:, s, tm * 128:(tm + 1) * 128], in_=tr_ps)

    ph12.close()

    # ---------------- phase 3: attention ----------------
    tail = ExitStack()
    ph3 = ExitStack()
    wc_pool = tail.enter_context(tc.tile_pool(name="wc", bufs=1))
    yt_pool = tail.enter_context(tc.tile_pool(name="yt", bufs=1))
    pt_pool = ph3.enter_context(tc.tile_pool(name="pt", bufs=6))
    lw_pool = ph3.enter_context(tc.tile_pool(name="lwork", bufs=2))
    ps_s = ph3.enter_context(tc.tile_pool(name="pss", bufs=3, space="PSUM"))
    ps_y = ph3.enter_context(tc.tile_pool(name="psy", bufs=2, space="PSUM"))
    ps_l = ph3.enter_context(tc.tile_pool(name="psl", bufs=2, space="PSUM"))

    wc_sb = wc_pool.tile([128, QH, 8, 512], F32R)
    for h in range(QH):
        for oc in range(8):
            nc.sync.dma_start(out=wc_sb[:, h, oc, :],
                              in_=wc[h * 128:(h + 1) * 128,
                                     oc * 512:(oc + 1) * 512])
    yT = yt_pool.tile([128, QH, T], F32R)


    def _attn_epilogue(h, qb, y_ps, l_ps):
        # normalize: yT[:, h, qb] = y_ps * (1/l) + bv
        l_row = lw_pool.tile([1, 512], F32R)
        nc.vector.tensor_copy(out=l_row, in_=l_ps)
        l_bc_ps = ps_s.tile([128, 512], F32, tag="s_ps")
        nc.tensor.matmul(l_bc_ps, ones_sb[0:1, :], l_row,
                         start=True, stop=True, skip_group_check=True)
        linv = lw_pool.tile([128, 512], F32)
        nc.vector.reciprocal(out=linv, in_=l_bc_ps)
        yn = lw_pool.tile([128, 512], F32)
        nc.vector.tensor_mul(yn, y_ps, linv)
        nc.scalar.activation(out=yT[:, h, qb * 512:(qb + 1) * 512],
                             in_=yn,
                             func=mybir.ActivationFunctionType.Identity,
                             bias=bv_sb, scale=1.0)

    pending = None
    for h in range(QH):
        for qb in range(NQB):
            nkc = 4 * (qb + 1)
            y_ps = ps_y.tile([128, 512], F32)
            l_ps = ps_l.tile([1, 512], F32)
            for kc in range(nkc):
                s_ps = ps_s.tile([128, 512], F32, tag="s_ps")
                nc.tensor.matmul(s_ps,
                                 qkT[:, QH, kc * 128:(kc + 1) * 128],
                                 qkT[:, h, qb * 512:(qb + 1) * 512],
                                 start=True, stop=True, skip_group_check=True)
                pt = pt_pool.tile([128, 512], F32R)
                nc.scalar.activation(out=pt, in_=s_ps,
                                     func=mybir.ActivationFunctionType.Exp,
                                     scale=INV_SQRT_HS)
                if kc >= 4 * qb:
                    nc.vector.tensor_mul(pt, pt, mask_sb[:, kc - 4 * qb, :])
                nc.tensor.matmul(y_ps, v_sb[:, kc, :], pt,
                                 start=(kc == 0), stop=(kc == nkc - 1),
                                 skip_group_check=True)
                nc.tensor.matmul(l_ps, ones_sb[:, 0:1], pt,
                                 start=(kc == 0), stop=(kc == nkc - 1),
                                 skip_group_check=True)
                if kc == 0 and pending is not None:
                    _attn_epilogue(*pending)   # prev group's epilogue overlaps
                    pending = None
            pending = (h, qb, y_ps, l_ps)
    _attn_epilogue(*pending)

    ph3.close()

    # ---------------- phase 4: c_proj partial ----------------
    ph4 = ExitStack()
    out_pool = ph4.enter_context(tc.tile_pool(name="outsb", bufs=2))
    act_scratch_pool = ph4.enter_context(tc.tile_pool(name="actscr", bufs=1))
    ps_o = ph4.enter_context(tc.tile_pool(name="pso", bufs=3, space="PSUM"))
    act_scratch = act_scratch_pool.tile([1, 4], F32)

    for tm in range(NTM):
        out_sb = out_pool.tile([128, C], F32)
        for oc in range(8):
            o_ps = ps_o.tile([128, 512], F32)
            for h in range(QH):
                nc.tensor.matmul(o_ps, yT[:, h, tm * 128:(tm + 1) * 128],
                                 wc_sb[:, h, oc, :],
                                 start=(h == 0), stop=(h == QH - 1),
                                 skip_group_check=True)
            nc.vector.tensor_copy(out=out_sb[:, oc * 512:(oc + 1) * 512],
                                  in_=o_ps)
        # ACT runway then output DMA from ACT (producer-side trigger)
        nc.scalar.copy(out=act_scratch[0:1, 0:1], in_=out_sb[0:1, 0:1])
        nc.scalar.dma_start(out=out[tm * 128:(tm + 1) * 128, :], in_=out_sb)

    ph4.close()
    tail.close()
    persist.close()


# ---------------------------------------------------------------- host side

def _rope_cache_np(seq_len, dim):
    inv_freq = 1.0 / (SCALE * BASE ** (np.arange(0, dim, 2, dtype=np.float32) / dim))
    t = np.arange(seq_len, dtype=np.float32)
    freqs = np.outer(t, inv_freq).astype(np.float32)
    emb = np.concatenate([freqs, freqs], axis=-1)
    return np.cos(emb).astype(np.float32), np.sin(emb).astype(np.float32)


_CACHE = {}


def _get_nc():
    if "nc" not in _CACHE:
        _CACHE["nc"] = _build_nc()
    return _CACHE["nc"]


def kernel(q_x, Wq, bq, Wk, bk, Wv, bv, Wc, bc, _trace=False):
    q_x = np.asarray(q_x, dtype=np.float32)
    Wq = np.asarray(Wq, dtype=np.float32)
    Wk = np.asarray(Wk, dtype=np.float32)
    Wv = np.asarray(Wv, dtype=np.float32)
    Wc = np.asarray(Wc, dtype=np.float32)
    bq = np.asarray(bq, dtype=np.float32)
    bv = np.asarray(bv, dtype=np.float32)
    bc = np.asarray(bc, dtype=np.float32)
    # NOTE: bk is exactly softmax-invariant (adds a per-query constant to all
    # scores) so it is dropped on device.

    x = q_x.reshape(T, C)
    xt = np.ascontiguousarray(x.T)                       # [C, T]

    cos, sin = _rope_cache_np(T, HS)                     # [T, 128]
    sn_half = np.ascontiguousarray(sin[:, :HS // 2])     # [T, 64]

    # causal 0/1 masks for the 4 diagonal offsets
    masks = np.zeros((4 * 128, 512), dtype=np.float32)
    dk = np.arange(128)[:, None]
    dq = np.arange(512)[None, :]
    for o in range(4):
        masks[o * 128:(o + 1) * 128] = (dk + o * 128 <= dq).astype(np.float32)

    ones_t = np.ones((128, 128), dtype=np.float32)

    in_maps = []
    for c in range(NCORES):
        wq_c = Wq[c * DQ:(c + 1) * DQ, :]                # [512, C]
        wk_c = Wk[c * HS:(c + 1) * HS, :]                # [128, C]
        wv_c = Wv[c * HS:(c + 1) * HS, :]
        wqkv = np.ascontiguousarray(
            np.concatenate([wq_c, wk_c, wv_c], axis=0).T)  # [C, 768]
        wc_c = np.ascontiguousarray(Wc[:, c * DQ:(c + 1) * DQ].T)  # [512, C]
        bq_bc = np.broadcast_to(bq[c * DQ:(c + 1) * DQ], (128, DQ)).copy()
        bv_col = bv[c * HS:(c + 1) * HS].reshape(128, 1).copy()
        in_maps.append({
            "xt": xt, "wqkv": wqkv, "wc": wc_c, "cs": cos, "sn": sn_half,
            "masks": masks, "ones_t": ones_t, "bqbc": bq_bc, "bvcol": bv_col,
            "ident": np.eye(128, dtype=np.float32),
        })

    nc = _get_nc()
    res = run_bass_kernel_spmd(nc, in_maps, core_ids=list(range(NCORES)),
                               trace=_trace)
    acc = np.zeros((T, C), dtype=np.float64)
    for c in range(NCORES):
        acc += res.results[c]["out"].astype(np.float64)
    out = (acc + bc.astype(np.float64)).astype(np.float32)
    if _trace:
        _CACHE["last_exec_time_ns"] = res.exec_time_ns
        _CACHE["last_results"] = res
    return out.reshape(B, T, C)



# revision 23
# speedup vs baseline: 2.0184x; 2.0184x over previous
"""Trainium2 Bass kernel for a GQA attention block (B=1, T=2048, C=4096,
NH=32, NKV=8, HS=128), tensor-parallel over heads across 8 NeuronCores.

Per core c: 4 query heads (4c..4c+3) and 1 KV head (c). Everything on the PE
path is fp16 (same PE throughput as fp32r, half the LDWEIGHTS time, half the
DMA bytes, 2x DVE modes, ~16x less rounding than bf16):

  - projections W-stationary: out = W^T-chunk stationary, x^T moving ->
    q^T/k^T/v^T [HS, T] directly (no per-tile PE transposes); bias fused
    into the ACT PSUM->SBUF drain.
  - RoPE rotate-half via a constant +-1 permutation matmul on PE (handles
    the cross-partition shuffle), then 3 DVE ops (mul/mul/add) per chunk.
  - attention: S^T = k-chunk^T q [keys, queries] -> ACT exp(s/sqrt(d) - 8)
    (shift keeps fp16 in range; cancels in normalization) -> diagonal-tile
    triangle mask multiply -> y natural [128q, 129] with a ones-column
    appended to V so the softmax denominator accumulates for free ->
    per-partition reciprocal [128,1] + scale -> PE transpose to y^T.
    Causal trim: no upper-triangle tiles are computed.
  - c_proj W-stationary producing out^T [C, T] fp16 partials (host
    transposes and sums across cores = the TP all-reduce).
  - ~50 batched DMA instructions total (host pre-arranges every operand so
    each DMA is a [128, contiguous-bytes] blit).
"""
import sys

sys.path.insert(0, "/opt/trn_rl_repo")

import numpy as np

from contextlib import ExitStack

import concourse.bass as bass
import concourse.mybir as mybir
import concourse.tile as tile
from concourse.bass_utils import run_bass_kernel_spmd

# ---------------------------------------------------------------- constants
B, T, C = 1, 2048, 4096
NH, NKV, HS = 32, 8, 128
NCORES = 8
QH = NH // NCORES          # 4 query heads per core
DQ = QH * HS               # 512
NKC = C // 128             # 32 contraction chunks
BASE, SCALE = 10000.0, 1.0
INV_SQRT_HS = 1.0 / float(np.sqrt(HS))
EXP_SHIFT = 8.0
KIDX = 4                   # k's surface index in qkT / wqkv chunk order

F32 = mybir.dt.float32
F16 = mybir.dt.float16
IDENT = mybir.ActivationFunctionType.Identity
EXP = mybir.ActivationFunctionType.Exp

# ------------------------------------------------------- wait legalization
_TAIL_RUNWAY = 48


def _legalize_waits(nc):
    """walrus (this toolchain) allows ONE sync wait per ISA instruction.
    Split excess waits off onto standalone EventSemaphore instructions
    inserted immediately before the offender (same engine stream order)."""
    n_split = 0
    for bb in nc.m.functions[0].blocks:
        insts = bb.instructions
        if not any(i.sync_info and i.sync_info.on_wait and
                   len(i.sync_info.on_wait) > (0 if type(i).__name__ == "InstISA" else 1)
                   for i in insts):
            continue
        new_list = []
        for inst in insts:
            si = inst.sync_info
            is_raw_isa = type(inst).__name__ == "InstISA"
            keep_n = 0 if is_raw_isa else 1
            if si and si.on_wait and len(si.on_wait) > keep_n:
                waits = list(si.on_wait)
                split_off = waits if is_raw_isa else waits[:-1]
                for w in split_off:
                    ev = mybir.InstNoOp(
                        name=f"legal-wait-{nc.next_id()}",
                        ins=[], outs=[], engine=inst.engine,
                        bass_nofuse=True,
                        sync_info=mybir.SyncInfo(on_wait=[w], on_update=[]))
                    nc.register_instruction(ev, overwrite=True)
                    new_list.append(ev)
                    n_split += 1
                inst.sync_info = mybir.SyncInfo(
                    on_wait=[] if is_raw_isa else [waits[-1]],
                    on_update=list(si.on_update))
            new_list.append(inst)
        bb.instructions = new_list
    return n_split


def _audit(nc):
    bad = []
    for bb in nc.m.functions[0].blocks:
        for inst in bb.instructions:
            si = inst.sync_info
            if si and si.on_wait and len(si.on_wait) > 1:
                bad.append((type(inst).__name__, inst.name, str(inst.engine),
                            len(si.on_wait)))
    return bad


class _TailRunwayPatch:
    """Plant runway nops on SP right before Tile's tail drain so the drain's
    many queue waits can be redistributed by _legalize_waits."""

    def __enter__(self):
        self.orig = tile.TileContext._drain_and_barrier
        orig = self.orig

        def patched(tc_self, tick_clock, wait_clock):
            for _ in range(_TAIL_RUNWAY):
                tc_self.nc.sync.nop(nofuse=True)
            return orig(tc_self, tick_clock, wait_clock)

        tile.TileContext._drain_and_barrier = patched
        return self

    def __exit__(self, *a):
        tile.TileContext._drain_and_barrier = self.orig


# ---------------------------------------------------------------- builder

def _build_nc(debug=False):
    nc = bass.Bass(trn_type="TRN2")

    xq = nc.dram_tensor("xq", [128, 4, NKC, 512], F16, kind="ExternalInput")
    wqkv = nc.dram_tensor("wqkv", [128, 6, NKC, 128], F16, kind="ExternalInput")
    wcc = nc.dram_tensor("wcc", [128, QH, 32, 128], F16, kind="ExternalInput")
    cs = nc.dram_tensor("cs", [128, T], F16, kind="ExternalInput")
    sn = nc.dram_tensor("sn", [128, T], F16, kind="ExternalInput")
    pmt = nc.dram_tensor("pmt", [128, 128], F16, kind="ExternalInput")
    idt = nc.dram_tensor("idt", [128, 128], F16, kind="ExternalInput")
    tri = nc.dram_tensor("tri", [128, 128], F16, kind="ExternalInput")
    bias = nc.dram_tensor("bias", [128, 6], F32, kind="ExternalInput")
    otT = nc.dram_tensor("otT", [32, 128, 4, 512], F16, kind="ExternalOutput")
    dbg = None
    if debug:
        dbg = {
            "dbg_qk": nc.dram_tensor("dbg_qk", [128, QH + 1, T], F16, kind="ExternalOutput"),
            "dbg_va": nc.dram_tensor("dbg_va", [128, 16, 129], F16, kind="ExternalOutput"),
            "dbg_yt": nc.dram_tensor("dbg_yt", [128, QH, T], F16, kind="ExternalOutput"),
        }

    with _TailRunwayPatch(), tile.TileContext(nc) as tc:
        _trace_body(nc, tc, xq, wqkv, wcc, cs, sn, pmt, idt, tri, bias, otT, dbg)

    _legalize_waits(nc)
    bad = _audit(nc)
    if bad:
        raise RuntimeError(f"multi-wait instructions remain: {bad[:10]}")
    return nc


def _trace_body(nc, tc, xq, wqkv, wcc, cs, sn, pmt, idt, tri, bias, otT, dbg=None):
    persist = ExitStack()

    # ---------------- persistent SBUF ----------------
    misc = persist.enter_context(tc.tile_pool(name="misc", bufs=1))
    qk_pool = persist.enter_context(tc.tile_pool(name="qkp", bufs=1))
    va_pool = persist.enter_context(tc.tile_pool(name="vap", bufs=1))
    yt_pool = persist.enter_context(tc.tile_pool(name="ytp", bufs=1))

    cs_sb = misc.tile([128, T], F16)
    nc.sync.dma_start(out=cs_sb, in_=cs[:, :])
    sn_sb = misc.tile([128, T], F16)
    nc.sync.dma_start(out=sn_sb, in_=sn[:, :])
    pm_sb = misc.tile([128, 128], F16)
    nc.sync.dma_start(out=pm_sb, in_=pmt[:, :])
    idt_sb = misc.tile([128, 128], F16)
    nc.sync.dma_start(out=idt_sb, in_=idt[:, :])
    tri_sb = misc.tile([128, 128], F16)
    nc.sync.dma_start(out=tri_sb, in_=tri[:, :])
    bias_sb = misc.tile([128, 6], F32)
    nc.sync.dma_start(out=bias_sb, in_=bias[:, :])

    qkT = qk_pool.tile([128, QH + 1, T], F16)      # q heads 0..3, k at 4
    v_aug = va_pool.tile([128, 16, 130], F16)      # [keys, kt, HS+ones]
    nc.vector.memset(v_aug[:, :, 128:129], 1.0)
    yT = yt_pool.tile([128, QH, T], F16)

    pa = persist.enter_context(tc.tile_pool(name="pa", bufs=2, space="PSUM"))

    # attention pools (outlive phase 1, so opened first — LIFO pool stack)
    ph3s = ExitStack()
    pt_pool = ph3s.enter_context(tc.tile_pool(name="pt", bufs=3))
    yn_pool = ph3s.enter_context(tc.tile_pool(name="yn", bufs=3))
    li_pool = ph3s.enter_context(tc.tile_pool(name="li", bufs=3))
    ph3p = ExitStack()
    ps = ph3p.enter_context(tc.tile_pool(name="ps", bufs=2, space="PSUM"))
    py = ph3p.enter_context(tc.tile_pool(name="py", bufs=4, space="PSUM"))

    # ---------------- phase 1 pools ----------------
    ph1 = ExitStack()
    w_pool = ph1.enter_context(tc.tile_pool(name="wqkv", bufs=1))
    xt_pool = ph1.enter_context(tc.tile_pool(name="xt", bufs=2))
    qraw_pool = ph1.enter_context(tc.tile_pool(name="qraw", bufs=2))
    ta_pool = ph1.enter_context(tc.tile_pool(name="ta", bufs=2))
    vt_pool = ph1.enter_context(tc.tile_pool(name="vt", bufs=2))

    wqkv_sb = w_pool.tile([128, 6, NKC, 128], F16)
    # chunk order: k, v first (unblocks attention), then q heads
    order = [KIDX, 5, 0, 1, 2, 3]
    for c in order:
        nc.sync.dma_start(out=wqkv_sb[:, c, :, :], in_=wqkv[:, c, :, :])

    def proj_quarter(qt):
        tsl = slice(qt * 512, (qt + 1) * 512)
        xt_sb = xt_pool.tile([128, NKC, 512], F16)
        nc.sync.dma_start(out=xt_sb, in_=xq[:, qt, :, :])
        pending = None
        for c in order:
            acc = pa.tile([128, 512], F32, tag="pa")
            for kc in range(NKC):
                nc.tensor.matmul(acc, wqkv_sb[:, c, kc, :], xt_sb[:, kc, :],
                                 start=(kc == 0), stop=(kc == NKC - 1),
                                 skip_group_check=True)
            # previous chunk's PE epilogue lands after this chunk's matmuls
            # so its ACT-drain latency is hidden
            if pending is not None:
                _emit_pending(pending, qt, tsl)
            if c == 5:   # v: drain with bias, then transpose to natural
                vt_sb = vt_pool.tile([128, 512], F16)
                nc.scalar.activation(out=vt_sb, in_=acc, func=IDENT,
                                     bias=bias_sb[:, 4:5], scale=1.0)
                pending = ("v", vt_sb)
            else:        # q head c (or k): drain (+bias), RoPE
                qraw = qraw_pool.tile([128, 512], F16)
                if c == KIDX:
                    nc.scalar.copy(out=qraw, in_=acc)
                else:
                    nc.scalar.activation(out=qraw, in_=acc, func=IDENT,
                                         bias=bias_sb[:, c:c + 1], scale=1.0)
                pending = ("rope", c, qraw)
        _emit_pending(pending, qt, tsl)

    def _emit_pending(pending, qt, tsl):
        # PE work for the previous chunk, emitted after the next chunk's
        # accumulation matmuls so the ACT drain latency is hidden.
        if pending[0] == "v":
            vt_sb = pending[1]
            for i in range(4):
                kt = qt * 4 + i
                vtr = pa.tile([128, 128], F16, tag="pa")
                nc.tensor.matmul(vtr, vt_sb[:, i * 128:(i + 1) * 128], idt_sb,
                                 is_transpose=True, skip_group_check=True)
                nc.vector.tensor_copy(out=v_aug[:, kt, 0:128], in_=vtr)
        else:
            _, c, qraw = pending
            rot = pa.tile([128, 512], F32, tag="pa")
            nc.tensor.matmul(rot, pm_sb, qraw, start=True, stop=True,
                             skip_group_check=True)
            dst = qkT[:, c, tsl]
            ta = ta_pool.tile([128, 512], F16)
            nc.vector.tensor_mul(ta, rot, sn_sb[:, tsl])
            nc.vector.tensor_mul(dst, qraw, cs_sb[:, tsl])
            nc.vector.tensor_add(dst, dst, ta)

    def attn_group(qb, h):
        nkc = 4 * qb + 4
        # two 2-qsub accumulators: each [2,130] f32 region fits one PSUM bank
        y_lo = py.tile([128, 2, 130], F32, tag="y")
        y_hi = py.tile([128, 2, 130], F32, tag="y")
        y_of = lambda qsub: (y_lo if qsub < 2 else y_hi)[:, qsub % 2, :]
        pts = {}

        def s_and_exp(kc):
            qs0 = max(0, kc - 4 * qb)
            w = 512 - qs0 * 128
            s_ps = ps.tile([128, 512], F32, tag="ps")
            nc.tensor.matmul(s_ps[:, 0:w],
                             qkT[:, KIDX, kc * 128:(kc + 1) * 128],
                             qkT[:, h, qb * 512 + qs0 * 128:(qb + 1) * 512],
                             start=True, stop=True, skip_group_check=True)
            pt = pt_pool.tile([128, 512], F16)
            nc.scalar.activation(out=pt[:, 0:w], in_=s_ps[:, 0:w], func=EXP,
                                 bias=bias_sb[:, 5:6], scale=INV_SQRT_HS)
            if kc >= 4 * qb:
                j = kc - 4 * qb - qs0   # diagonal tile, pt-local index
                nc.vector.tensor_mul(pt[:, j * 128:(j + 1) * 128],
                                     pt[:, j * 128:(j + 1) * 128], tri_sb)
            pts[kc] = (pt, qs0)

        def y_mms(kc):
            pt, qs0 = pts.pop(kc)
            for qsub in range(qs0, 4):
                # start=True clears has_written for the WHOLE 2KB psum bank,
                # so only the first matmul into each [2,130] tile may set it;
                # the sibling region's first write zero-overwrites via the
                # pending-zero mechanism.
                nc.tensor.matmul(y_of(qsub)[:, 0:129],
                                 pt[:, (qsub - qs0) * 128:(qsub - qs0 + 1) * 128],
                                 v_aug[:, kc, 0:129],
                                 start=(kc == 0 and qsub % 2 == 0),
                                 stop=(kc == 4 * qb + qsub),
                                 skip_group_check=True)

        s_and_exp(0)
        if nkc > 1:
            s_and_exp(1)
        for kc in range(nkc):
            if kc + 2 < nkc:
                s_and_exp(kc + 2)
            y_mms(kc)

        for qsub in range(4):
            linv = li_pool.tile([128, 1], F32)
            nc.vector.reciprocal(linv, y_of(qsub)[:, 128:129])
            yn = yn_pool.tile([128, 128], F16)
            nc.vector.tensor_scalar_mul(yn, y_of(qsub)[:, 0:128], linv)
            ytr = ps.tile([128, 128], F16, tag="ps")
            nc.tensor.matmul(ytr, yn, idt_sb, is_transpose=True,
                             skip_group_check=True)
            nc.vector.tensor_copy(
                out=yT[:, h, (qb * 4 + qsub) * 128:(qb * 4 + qsub + 1) * 128],
                in_=ytr)

    # ---------------- phases 1-3 interleaved ----------------
    for qt in range(4):
        proj_quarter(qt)
        for h in range(QH):
            attn_group(qt, h)

    ph1.close()

    if dbg is not None:
        nc.sync.dma_start(out=dbg["dbg_qk"][:, :, :], in_=qkT)
        nc.sync.dma_start(out=dbg["dbg_va"][:, :, :], in_=v_aug[:, :, 0:129])
        nc.sync.dma_start(out=dbg["dbg_yt"][:, :, :], in_=yT)

    # ---------------- phase 4: c_proj -> out^T ----------------
    tail = ExitStack()
    wc_pool = tail.enter_context(tc.tile_pool(name="wc", bufs=1))
    wc_sb = wc_pool.tile([128, QH, 32, 128], F16)
    nc.sync.dma_start(out=wc_sb, in_=wcc[:, :, :, :])

    ph3p.close()

    ph4 = ExitStack()
    oc_pool = ph4.enter_context(tc.tile_pool(name="oc", bufs=3))
    pc = ph4.enter_context(tc.tile_pool(name="pc", bufs=2, space="PSUM"))

    for cb in range(32):
        oc_sb = oc_pool.tile([128, 4, 512], F16)
        for quad in range(4):
            occ = pc.tile([128, 512], F32)
            for hd in range(QH):
                nc.tensor.matmul(occ, wc_sb[:, hd, cb, :],
                                 yT[:, hd, quad * 512:(quad + 1) * 512],
                                 start=(hd == 0), stop=(hd == QH - 1),
                                 skip_group_check=True)
            nc.scalar.copy(out=oc_sb[:, quad, :], in_=occ)
        nc.sync.dma_start(out=otT[cb, :, :, :], in_=oc_sb)

    ph4.close()
    tail.close()
    ph3s.close()
    persist.close()


# ---------------------------------------------------------------- host side

def _rope_T_np(seq_len, hs):
    inv_freq = 1.0 / (SCALE * BASE ** (np.arange(0, hs, 2, dtype=np.float64) / hs))
    freqs = np.outer(inv_freq, np.arange(seq_len, dtype=np.float64))  # [64, T]
    emb = np.concatenate([freqs, freqs], axis=0)                      # [128, T]
    return np.cos(emb).astype(np.float16), np.sin(emb).astype(np.float16)


_CACHE = {}


def _get_nc():
    if "nc" not in _CACHE:
        _CACHE["nc"] = _build_nc()
    return _CACHE["nc"]


def kernel(q_x, Wq, bq, Wk, bk, Wv, bv, Wc, bc, _trace=False):
    q_x = np.asarray(q_x, dtype=np.float32)
    Wq = np.asarray(Wq, dtype=np.float32)
    Wk = np.asarray(Wk, dtype=np.float32)
    Wv = np.asarray(Wv, dtype=np.float32)
    Wc = np.asarray(Wc, dtype=np.float32)
    bq = np.asarray(bq, dtype=np.float32)
    bv = np.asarray(bv, dtype=np.float32)
    bc = np.asarray(bc, dtype=np.float32)
    # NOTE: bk is exactly softmax-invariant (adds a per-query constant to all
    # scores) so it is dropped on device.

    x = q_x.reshape(T, C)
    xT = np.ascontiguousarray(x.T).astype(np.float16)          # [C, T]
    # xq[p, qt, kc, j] = xT[kc*128+p, qt*512+j]
    xq = np.ascontiguousarray(
        xT.reshape(NKC, 128, 4, 512).transpose(1, 2, 0, 3))

    cosT, snT = _rope_T_np(T, HS)

    pm = np.zeros((128, 128), np.float16)
    for m in range(64):
        pm[m + 64, m] = -1.0
        pm[m, m + 64] = 1.0
    ident = np.eye(128, dtype=np.float16)
    tri = (np.arange(128)[:, None] <= np.arange(128)[None, :]).astype(np.float16)

    in_maps = []
    for c in range(NCORES):
        wq_c = Wq[c * DQ:(c + 1) * DQ, :]                # [512, C]
        wk_c = Wk[c * HS:(c + 1) * HS, :]                # [128, C]
        wv_c = Wv[c * HS:(c + 1) * HS, :]
        wcat = np.concatenate([wq_c, wk_c, wv_c], axis=0).astype(np.float16)
        # wqkv[p, ch, kc, j] = wcat[ch*128+j, kc*128+p]
        wqkv_a = np.ascontiguousarray(
            wcat.reshape(6, 128, NKC, 128).transpose(3, 0, 2, 1))
        wcT = np.ascontiguousarray(Wc[:, c * DQ:(c + 1) * DQ].T).astype(np.float16)
        # wcc[p, hd, cb, j] = wcT[hd*128+p, cb*128+j]
        wcc_a = np.ascontiguousarray(
            wcT.reshape(QH, 128, 32, 128).transpose(1, 0, 2, 3))
        bias_a = np.zeros((128, 6), np.float32)
        for h in range(QH):
            bias_a[:, h] = bq[c * DQ + h * HS: c * DQ + (h + 1) * HS]
        bias_a[:, 4] = bv[c * HS:(c + 1) * HS]
        bias_a[:, 5] = -EXP_SHIFT
        in_maps.append({
            "xq": xq, "wqkv": wqkv_a, "wcc": wcc_a, "cs": cosT, "sn": snT,
            "pmt": pm, "idt": ident, "tri": tri, "bias": bias_a,
        })

    nc = _get_nc()
    res = run_bass_kernel_spmd(nc, in_maps, core_ids=list(range(NCORES)),
                               trace=_trace)
    acc = np.zeros((C, T), dtype=np.float64)
    for c in range(NCORES):
        acc += res.results[c]["otT"].reshape(C, T).astype(np.float64)
    out = (acc.T + bc.astype(np.float64)[None, :]).astype(np.float32)
    if _trace:
        _CACHE["last_exec_time_ns"] = res.exec_time_ns
        _CACHE["last_results"] = res
    return out.reshape(B, T, C)


# revision 28
# speedup vs baseline: 2.0858x; 1.0334x over previous
"""Trainium2 Bass kernel for a GQA attention block (B=1, T=2048, C=4096,
NH=32, NKV=8, HS=128), tensor-parallel over heads across 8 NeuronCores.

Per core c: 4 query heads (4c..4c+3) and 1 KV head (c). Everything on the PE
path is fp16 (same PE throughput as fp32r, half the LDWEIGHTS time, half the
DMA bytes, 2x DVE modes, ~16x less rounding than bf16):

  - projections W-stationary: out = W^T-chunk stationary, x^T moving ->
    q^T/k^T/v^T [HS, T] directly (no per-tile PE transposes); bias fused
    into the ACT PSUM->SBUF drain.
  - RoPE rotate-half via a constant +-1 permutation matmul on PE (handles
    the cross-partition shuffle), then 3 DVE ops (mul/mul/add) per chunk.
  - attention: S^T = k-chunk^T q [keys, queries] -> ACT exp(s/sqrt(d) - 8)
    (shift keeps fp16 in range; cancels in normalization) -> diagonal-tile
    triangle mask multiply -> y natural [128q, 129] with a ones-column
    appended to V so the softmax denominator accumulates for free ->
    per-partition reciprocal [128,1] + scale -> PE transpose to y^T.
    Causal trim: no upper-triangle tiles are computed.
  - c_proj W-stationary producing out^T [C, T] fp16 partials (host
    transposes and sums across cores = the TP all-reduce).
  - ~50 batched DMA instructions total (host pre-arranges every operand so
    each DMA is a [128, contiguous-bytes] blit).
"""
import sys

sys.path.insert(0, "/opt/trn_rl_repo")

import numpy as np

from contextlib import ExitStack

import concourse.bass as bass
import concourse.mybir as mybir
import concourse.tile as tile
from concourse.bass_utils import run_bass_kernel_spmd

# ---------------------------------------------------------------- constants
B, T, C = 1, 2048, 4096
NH, NKV, HS = 32, 8, 128
NCORES = 8
QH = NH // NCORES          # 4 query heads per core
DQ = QH * HS               # 512
NKC = C // 128             # 32 contraction chunks
BASE, SCALE = 10000.0, 1.0
INV_SQRT_HS = 1.0 / float(np.sqrt(HS))
EXP_SHIFT = 8.0
KIDX = 4                   # k's surface index in qkT / wqkv chunk order

F32 = mybir.dt.float32
F16 = mybir.dt.float16
IDENT = mybir.ActivationFunctionType.Identity
EXP = mybir.ActivationFunctionType.Exp

# ------------------------------------------------------- wait legalization
_TAIL_RUNWAY = 48


def _legalize_waits(nc):
    """walrus (this toolchain) allows ONE sync wait per ISA instruction.
    Split excess waits off onto standalone EventSemaphore instructions
    inserted immediately before the offender (same engine stream order)."""
    n_split = 0
    for bb in nc.m.functions[0].blocks:
        insts = bb.instructions
        if not any(i.sync_info and i.sync_info.on_wait and
                   len(i.sync_info.on_wait) > (0 if type(i).__name__ == "InstISA" else 1)
                   for i in insts):
            continue
        new_list = []
        for inst in insts:
            si = inst.sync_info
            is_raw_isa = type(inst).__name__ == "InstISA"
            keep_n = 0 if is_raw_isa else 1
            if si and si.on_wait and len(si.on_wait) > keep_n:
                waits = list(si.on_wait)
                split_off = waits if is_raw_isa else waits[:-1]
                for w in split_off:
                    ev = mybir.InstNoOp(
                        name=f"legal-wait-{nc.next_id()}",
                        ins=[], outs=[], engine=inst.engine,
                        bass_nofuse=True,
                        sync_info=mybir.SyncInfo(on_wait=[w], on_update=[]))
                    nc.register_instruction(ev, overwrite=True)
                    new_list.append(ev)
                    n_split += 1
                inst.sync_info = mybir.SyncInfo(
                    on_wait=[] if is_raw_isa else [waits[-1]],
                    on_update=list(si.on_update))
            new_list.append(inst)
        bb.instructions = new_list
    return n_split


def _audit(nc):
    bad = []
    for bb in nc.m.functions[0].blocks:
        for inst in bb.instructions:
            si = inst.sync_info
            if si and si.on_wait and len(si.on_wait) > 1:
                bad.append((type(inst).__name__, inst.name, str(inst.engine),
                            len(si.on_wait)))
    return bad


class _TailRunwayPatch:
    """Plant runway nops on SP right before Tile's tail drain so the drain's
    many queue waits can be redistributed by _legalize_waits."""

    def __enter__(self):
        self.orig = tile.TileContext._drain_and_barrier
        orig = self.orig

        def patched(tc_self, tick_clock, wait_clock):
            for _ in range(_TAIL_RUNWAY):
                tc_self.nc.sync.nop(nofuse=True)
            return orig(tc_self, tick_clock, wait_clock)

        tile.TileContext._drain_and_barrier = patched
        return self

    def __exit__(self, *a):
        tile.TileContext._drain_and_barrier = self.orig


# ---------------------------------------------------------------- builder

def _build_nc(debug=False):
    nc = bass.Bass(trn_type="TRN2")

    xq = nc.dram_tensor("xq", [128, 4, NKC, 512], F16, kind="ExternalInput")
    wqkv = nc.dram_tensor("wqkv", [128, 6, NKC, 128], F16, kind="ExternalInput")
    wcc = nc.dram_tensor("wcc", [128, QH, 32, 128], F16, kind="ExternalInput")
    cs = nc.dram_tensor("cs", [128, T], F16, kind="ExternalInput")
    sn = nc.dram_tensor("sn", [128, T], F16, kind="ExternalInput")
    pmt = nc.dram_tensor("pmt", [128, 128], F16, kind="ExternalInput")
    idt = nc.dram_tensor("idt", [128, 128], F16, kind="ExternalInput")
    tri = nc.dram_tensor("tri", [128, 128], F16, kind="ExternalInput")
    bias = nc.dram_tensor("bias", [128, 6], F32, kind="ExternalInput")
    otT = nc.dram_tensor("otT", [32, 128, 4, 512], F16, kind="ExternalOutput")
    dbg = None
    if debug:
        dbg = {
            "dbg_qk": nc.dram_tensor("dbg_qk", [128, QH + 1, T], F16, kind="ExternalOutput"),
            "dbg_va": nc.dram_tensor("dbg_va", [128, 16, 129], F16, kind="ExternalOutput"),
            "dbg_yt": nc.dram_tensor("dbg_yt", [128, QH, T], F16, kind="ExternalOutput"),
        }

    with _TailRunwayPatch(), tile.TileContext(nc) as tc:
        _trace_body(nc, tc, xq, wqkv, wcc, cs, sn, pmt, idt, tri, bias, otT, dbg)

    _legalize_waits(nc)
    bad = _audit(nc)
    if bad:
        raise RuntimeError(f"multi-wait instructions remain: {bad[:10]}")
    return nc


def _trace_body(nc, tc, xq, wqkv, wcc, cs, sn, pmt, idt, tri, bias, otT, dbg=None):
    persist = ExitStack()

    # ---------------- persistent SBUF ----------------
    misc = persist.enter_context(tc.tile_pool(name="misc", bufs=1))
    qk_pool = persist.enter_context(tc.tile_pool(name="qkp", bufs=1))
    va_pool = persist.enter_context(tc.tile_pool(name="vap", bufs=1))
    yt_pool = persist.enter_context(tc.tile_pool(name="ytp", bufs=1))

    cs_sb = misc.tile([128, T], F16)
    sn_sb = misc.tile([128, T], F16)
    pm_sb = misc.tile([128, 128], F16)
    idt_sb = misc.tile([128, 128], F16)
    tri_sb = misc.tile([128, 128], F16)
    bias_sb = misc.tile([128, 6], F32)

    qkT = qk_pool.tile([128, QH + 1, T], F16)      # q heads 0..3, k at 4
    v_aug = va_pool.tile([128, 16, 130], F16)      # [keys, kt, HS+ones]
    nc.vector.memset(v_aug[:, :, 128:129], 1.0)
    yT = yt_pool.tile([128, QH, T], F16)

    pa = persist.enter_context(tc.tile_pool(name="pa", bufs=2, space="PSUM"))

    # attention pools (outlive phase 1, so opened first — LIFO pool stack)
    ph3s = ExitStack()
    pt_pool = ph3s.enter_context(tc.tile_pool(name="pt", bufs=3))
    yn_pool = ph3s.enter_context(tc.tile_pool(name="yn", bufs=3))
    li_pool = ph3s.enter_context(tc.tile_pool(name="li", bufs=3))
    ph3p = ExitStack()
    ps = ph3p.enter_context(tc.tile_pool(name="ps", bufs=2, space="PSUM"))
    py = ph3p.enter_context(tc.tile_pool(name="py", bufs=4, space="PSUM"))

    # ---------------- phase 1 pools ----------------
    ph1 = ExitStack()
    w_pool = ph1.enter_context(tc.tile_pool(name="wqkv", bufs=1))
    xt_pool = ph1.enter_context(tc.tile_pool(name="xt", bufs=2))
    qraw_pool = ph1.enter_context(tc.tile_pool(name="qraw", bufs=2))
    ta_pool = ph1.enter_context(tc.tile_pool(name="ta", bufs=2))
    vt_pool = ph1.enter_context(tc.tile_pool(name="vt", bufs=2))

    wqkv_sb = w_pool.tile([128, 6, NKC, 128], F16)
    # chunk order: k, v first (unblocks attention), then q heads
    order = [KIDX, 5, 0, 1, 2, 3]
    # startup-critical DMA order: chunk-k weights, then quarter-0 x in 8-kc
    # slices (first proj matmul starts after ~2MB, not ~13MB), then the rest
    nc.sync.dma_start(out=wqkv_sb[:, KIDX, :, :], in_=wqkv[:, KIDX, :, :])
    xt0_sb = xt_pool.tile([128, NKC, 512], F16, tag="xt")
    for g in range(4):
        nc.sync.dma_start(out=xt0_sb[:, g * 8:(g + 1) * 8, :],
                          in_=xq[:, 0, g * 8:(g + 1) * 8, :])
    for c in order[1:]:
        nc.sync.dma_start(out=wqkv_sb[:, c, :, :], in_=wqkv[:, c, :, :])
    nc.sync.dma_start(out=cs_sb, in_=cs[:, :])
    nc.sync.dma_start(out=sn_sb, in_=sn[:, :])
    nc.sync.dma_start(out=pm_sb, in_=pmt[:, :])
    nc.sync.dma_start(out=idt_sb, in_=idt[:, :])
    nc.sync.dma_start(out=tri_sb, in_=tri[:, :])
    nc.sync.dma_start(out=bias_sb, in_=bias[:, :])

    def proj_quarter(qt):
        tsl = slice(qt * 512, (qt + 1) * 512)
        if qt == 0:
            xt_sb = xt0_sb
        else:
            xt_sb = xt_pool.tile([128, NKC, 512], F16, tag="xt")
            nc.sync.dma_start(out=xt_sb, in_=xq[:, qt, :, :])
        pending = None
        for c in order:
            acc = pa.tile([128, 512], F32, tag="pa")
            for kc in range(NKC):
                nc.tensor.matmul(acc, wqkv_sb[:, c, kc, :], xt_sb[:, kc, :],
                                 start=(kc == 0), stop=(kc == NKC - 1),
                                 skip_group_check=True)
            # previous chunk's PE epilogue lands after this chunk's matmuls
            # so its ACT-drain latency is hidden
            if pending is not None:
                _emit_pending(pending, qt, tsl)
            if c == 5:   # v: drain with bias, then transpose to natural
                vt_sb = vt_pool.tile([128, 512], F16)
                nc.scalar.activation(out=vt_sb, in_=acc, func=IDENT,
                                     bias=bias_sb[:, 4:5], scale=1.0)
                pending = ("v", vt_sb)
            else:        # q head c (or k): drain (+bias), RoPE
                qraw = qraw_pool.tile([128, 512], F16)
                if c == KIDX:
                    nc.scalar.copy(out=qraw, in_=acc)
                else:
                    nc.scalar.activation(out=qraw, in_=acc, func=IDENT,
                                         bias=bias_sb[:, c:c + 1], scale=1.0)
                pending = ("rope", c, qraw)
        _emit_pending(pending, qt, tsl)

    def _emit_pending(pending, qt, tsl):
        # PE work for the previous chunk, emitted after the next chunk's
        # accumulation matmuls so the ACT drain latency is hidden.
        if pending[0] == "v":
            vt_sb = pending[1]
            for i in range(4):
                kt = qt * 4 + i
                vtr = pa.tile([128, 128], F16, tag="pa")
                nc.tensor.matmul(vtr, vt_sb[:, i * 128:(i + 1) * 128], idt_sb,
                                 is_transpose=True, skip_group_check=True)
                nc.vector.tensor_copy(out=v_aug[:, kt, 0:128], in_=vtr)
        else:
            _, c, qraw = pending
            rot = pa.tile([128, 512], F32, tag="pa")
            nc.tensor.matmul(rot, pm_sb, qraw, start=True, stop=True,
                             skip_group_check=True)
            dst = qkT[:, c, tsl]
            ta = ta_pool.tile([128, 512], F16)
            nc.vector.tensor_mul(ta, rot, sn_sb[:, tsl])
            nc.vector.tensor_mul(dst, qraw, cs_sb[:, tsl])
            nc.vector.tensor_add(dst, dst, ta)

    def attn_group(qb, h):
        nkc = 4 * qb + 4
        # two 2-qsub accumulators: each [2,130] f32 region fits one PSUM bank
        y_lo = py.tile([128, 2, 130], F32, tag="y")
        y_hi = py.tile([128, 2, 130], F32, tag="y")
        y_of = lambda qsub: (y_lo if qsub < 2 else y_hi)[:, qsub % 2, :]
        pts = {}

        def s_and_exp(kc):
            qs0 = max(0, kc - 4 * qb)
            w = 512 - qs0 * 128
            s_ps = ps.tile([128, 512], F32, tag="ps")
            nc.tensor.matmul(s_ps[:, 0:w],
                             qkT[:, KIDX, kc * 128:(kc + 1) * 128],
                             qkT[:, h, qb * 512 + qs0 * 128:(qb + 1) * 512],
                             start=True, stop=True, skip_group_check=True)
            pt = pt_pool.tile([128, 512], F16)
            nc.scalar.activation(out=pt[:, 0:w], in_=s_ps[:, 0:w], func=EXP,
                                 bias=bias_sb[:, 5:6], scale=INV_SQRT_HS)
            if kc >= 4 * qb:
                j = kc - 4 * qb - qs0   # diagonal tile, pt-local index
                nc.vector.tensor_mul(pt[:, j * 128:(j + 1) * 128],
                                     pt[:, j * 128:(j + 1) * 128], tri_sb)
            pts[kc] = (pt, qs0)

        def y_mms(kc):
            pt, qs0 = pts.pop(kc)
            for qsub in range(qs0, 4):
                # start=True clears has_written for the WHOLE 2KB psum bank,
                # so only the first matmul into each [2,130] tile may set it;
                # the sibling region's first write zero-overwrites via the
                # pending-zero mechanism.
                nc.tensor.matmul(y_of(qsub)[:, 0:129],
                                 pt[:, (qsub - qs0) * 128:(qsub - qs0 + 1) * 128],
                                 v_aug[:, kc, 0:129],
                                 start=(kc == 0 and qsub % 2 == 0),
                                 stop=(kc == 4 * qb + qsub),
                                 skip_group_check=True)

        s_and_exp(0)
        if nkc > 1:
            s_and_exp(1)
        for kc in range(nkc):
            if kc + 2 < nkc:
                s_and_exp(kc + 2)
            y_mms(kc)

        for qsub in range(4):
            linv = li_pool.tile([128, 1], F32)
            nc.vector.reciprocal(linv, y_of(qsub)[:, 128:129])
            yn = yn_pool.tile([128, 128], F16)
            nc.vector.tensor_scalar_mul(yn, y_of(qsub)[:, 0:128], linv)
            ytr = ps.tile([128, 128], F16, tag="ps")
            nc.tensor.matmul(ytr, yn, idt_sb, is_transpose=True,
                             skip_group_check=True)
            nc.vector.tensor_copy(
                out=yT[:, h, (qb * 4 + qsub) * 128:(qb * 4 + qsub + 1) * 128],
                in_=ytr)

    # ---------------- phases 1-3 interleaved ----------------
    for qt in range(4):
        proj_quarter(qt)
        for h in range(QH):
            attn_group(qt, h)

    ph1.close()

    if dbg is not None:
        nc.sync.dma_start(out=dbg["dbg_qk"][:, :, :], in_=qkT)
        nc.sync.dma_start(out=dbg["dbg_va"][:, :, :], in_=v_aug[:, :, 0:129])
        nc.sync.dma_start(out=dbg["dbg_yt"][:, :, :], in_=yT)

    # ---------------- phase 4: c_proj -> out^T ----------------
    tail = ExitStack()
    wc_pool = tail.enter_context(tc.tile_pool(name="wc", bufs=1))
    wc_sb = wc_pool.tile([128, QH, 32, 128], F16)
    nc.sync.dma_start(out=wc_sb, in_=wcc[:, :, :, :])

    ph3p.close()

    ph4 = ExitStack()
    oc_pool = ph4.enter_context(tc.tile_pool(name="oc", bufs=3))
    pc = ph4.enter_context(tc.tile_pool(name="pc", bufs=2, space="PSUM"))

    for cb in range(32):
        oc_sb = oc_pool.tile([128, 4, 512], F16)
        for quad in range(4):
            occ = pc.tile([128, 512], F32)
            for hd in range(QH):
                nc.tensor.matmul(occ, wc_sb[:, hd, cb, :],
                                 yT[:, hd, quad * 512:(quad + 1) * 512],
                                 start=(hd == 0), stop=(hd == QH - 1),
                                 skip_group_check=True)
            nc.scalar.copy(out=oc_sb[:, quad, :], in_=occ)
        nc.sync.dma_start(out=otT[cb, :, :, :], in_=oc_sb)

    ph4.close()
    tail.close()
    ph3s.close()
    persist.close()


# ---------------------------------------------------------------- host side

def _rope_T_np(seq_len, hs):
    inv_freq = 1.0 / (SCALE * BASE ** (np.arange(0, hs, 2, dtype=np.float64) / hs))
    freqs = np.outer(inv_freq, np.arange(seq_len, dtype=np.float64))  # [64, T]
    emb = np.concatenate([freqs, freqs], axis=0)                      # [128, T]
    return np.cos(emb).astype(np.float16), np.sin(emb).astype(np.float16)


_CACHE = {}


def _get_nc():
    if "nc" not in _CACHE:
        _CACHE["nc"] = _build_nc()
    return _CACHE["nc"]


def kernel(q_x, Wq, bq, Wk, bk, Wv, bv, Wc, bc, _trace=False):
    q_x = np.asarray(q_x, dtype=np.float32)
    Wq = np.asarray(Wq, dtype=np.float32)
    Wk = np.asarray(Wk, dtype=np.float32)
    Wv = np.asarray(Wv, dtype=np.float32)
    Wc = np.asarray(Wc, dtype=np.float32)
    bq = np.asarray(bq, dtype=np.float32)
    bv = np.asarray(bv, dtype=np.float32)
    bc = np.asarray(bc, dtype=np.float32)
    # NOTE: bk is exactly softmax-invariant (adds a per-query constant to all
    # scores) so it is dropped on device.

    x = q_x.reshape(T, C)
    xT = np.ascontiguousarray(x.T).astype(np.float16)          # [C, T]
    # xq[p, qt, kc, j] = xT[kc*128+p, qt*512+j]
    xq = np.ascontiguousarray(
        xT.reshape(NKC, 128, 4, 512).transpose(1, 2, 0, 3))

    cosT, snT = _rope_T_np(T, HS)

    pm = np.zeros((128, 128), np.float16)
    for m in range(64):
        pm[m + 64, m] = -1.0
        pm[m, m + 64] = 1.0
    ident = np.eye(128, dtype=np.float16)
    tri = (np.arange(128)[:, None] <= np.arange(128)[None, :]).astype(np.float16)

    in_maps = []
    for c in range(NCORES):
        wq_c = Wq[c * DQ:(c + 1) * DQ, :]                # [512, C]
        wk_c = Wk[c * HS:(c + 1) * HS, :]                # [128, C]
        wv_c = Wv[c * HS:(c + 1) * HS, :]
        wcat = np.concatenate([wq_c, wk_c, wv_c], axis=0).astype(np.float16)
        # wqkv[p, ch, kc, j] = wcat[ch*128+j, kc*128+p]
        wqkv_a = np.ascontiguousarray(
            wcat.reshape(6, 128, NKC, 128).transpose(3, 0, 2, 1))
        wcT = np.ascontiguousarray(Wc[:, c * DQ:(c + 1) * DQ].T).astype(np.float16)
        # wcc[p, hd, cb, j] = wcT[hd*128+p, cb*128+j]
        wcc_a = np.ascontiguousarray(
            wcT.reshape(QH, 128, 32, 128).transpose(1, 0, 2, 3))
        bias_a = np.zeros((128, 6), np.float32)
        for h in range(QH):
            bias_a[:, h] = bq[c * DQ + h * HS: c * DQ + (h + 1) * HS]
        bias_a[:, 4] = bv[c * HS:(c + 1) * HS]
        bias_a[:, 5] = -EXP_SHIFT
        in_maps.append({
            "xq": xq, "wqkv": wqkv_a, "wcc": wcc_a, "cs": cosT, "sn": snT,
            "pmt": pm, "idt": ident, "tri": tri, "bias": bias_a,
        })

    nc = _get_nc()
    res = run_bass_kernel_spmd(nc, in_maps, core_ids=list(range(NCORES)),
                               trace=_trace)
    acc = np.zeros((C, T), dtype=np.float64)
    for c in range(NCORES):
        acc += res.results[c]["otT"].reshape(C, T).astype(np.float64)
    out = (acc.T + bc.astype(np.float64)[None, :]).astype(np.float32)
    if _trace:
        _CACHE["last_exec_time_ns"] = res.exec_time_ns
        _CACHE["last_results"] = res
    return out.reshape(B, T, C)


# revision 31
# speedup vs baseline: 2.1085x; 1.0109x over previous
"""Trainium2 Bass kernel for a GQA attention block (B=1, T=2048, C=4096,
NH=32, NKV=8, HS=128), tensor-parallel over heads across 8 NeuronCores.

Per core c: 4 query heads (4c..4c+3) and 1 KV head (c). Everything on the PE
path is fp16 (same PE throughput as fp32r, half the LDWEIGHTS time, half the
DMA bytes, 2x DVE modes, ~16x less rounding than bf16):

  - projections W-stationary: out = W^T-chunk stationary, x^T moving ->
    q^T/k^T/v^T [HS, T] directly (no per-tile PE transposes); bias fused
    into the ACT PSUM->SBUF drain.
  - RoPE rotate-half via a constant +-1 permutation matmul on PE (handles
    the cross-partition shuffle), then 3 DVE ops (mul/mul/add) per chunk.
  - attention: S^T = k-chunk^T q [keys, queries] -> ACT exp(s/sqrt(d) - 8)
    (shift keeps fp16 in range; cancels in normalization) -> diagonal-tile
    triangle mask multiply -> y natural [128q, 129] with a ones-column
    appended to V so the softmax denominator accumulates for free ->
    per-partition reciprocal [128,1] + scale -> PE transpose to y^T.
    Causal trim: no upper-triangle tiles are computed.
  - c_proj W-stationary producing out^T [C, T] fp16 partials (host
    transposes and sums across cores = the TP all-reduce).
  - ~50 batched DMA instructions total (host pre-arranges every operand so
    each DMA is a [128, contiguous-bytes] blit).
"""
import sys

sys.path.insert(0, "/opt/trn_rl_repo")

import numpy as np

from contextlib import ExitStack

import concourse.bass as bass
import concourse.mybir as mybir
import concourse.tile as tile
from concourse.bass_utils import run_bass_kernel_spmd

# ---------------------------------------------------------------- constants
B, T, C = 1, 2048, 4096
NH, NKV, HS = 32, 8, 128
NCORES = 8
QH = NH // NCORES          # 4 query heads per core
DQ = QH * HS               # 512
NKC = C // 128             # 32 contraction chunks
BASE, SCALE = 10000.0, 1.0
INV_SQRT_HS = 1.0 / float(np.sqrt(HS))
EXP_SHIFT = 8.0
KIDX = 4                   # k's surface index in qkT / wqkv chunk order

F32 = mybir.dt.float32
F16 = mybir.dt.float16
IDENT = mybir.ActivationFunctionType.Identity
EXP = mybir.ActivationFunctionType.Exp

# ------------------------------------------------------- wait legalization
_TAIL_RUNWAY = 48


def _legalize_waits(nc):
    """walrus (this toolchain) allows ONE sync wait per ISA instruction.
    Split excess waits off onto standalone EventSemaphore instructions
    inserted immediately before the offender (same engine stream order)."""
    n_split = 0
    for bb in nc.m.functions[0].blocks:
        insts = bb.instructions
        if not any(i.sync_info and i.sync_info.on_wait and
                   len(i.sync_info.on_wait) > (0 if type(i).__name__ == "InstISA" else 1)
                   for i in insts):
            continue
        new_list = []
        for inst in insts:
            si = inst.sync_info
            is_raw_isa = type(inst).__name__ == "InstISA"
            keep_n = 0 if is_raw_isa else 1
            if si and si.on_wait and len(si.on_wait) > keep_n:
                waits = list(si.on_wait)
                split_off = waits if is_raw_isa else waits[:-1]
                for w in split_off:
                    ev = mybir.InstNoOp(
                        name=f"legal-wait-{nc.next_id()}",
                        ins=[], outs=[], engine=inst.engine,
                        bass_nofuse=True,
                        sync_info=mybir.SyncInfo(on_wait=[w], on_update=[]))
                    nc.register_instruction(ev, overwrite=True)
                    new_list.append(ev)
                    n_split += 1
                inst.sync_info = mybir.SyncInfo(
                    on_wait=[] if is_raw_isa else [waits[-1]],
                    on_update=list(si.on_update))
            new_list.append(inst)
        bb.instructions = new_list
    return n_split


def _audit(nc):
    bad = []
    for bb in nc.m.functions[0].blocks:
        for inst in bb.instructions:
            si = inst.sync_info
            if si and si.on_wait and len(si.on_wait) > 1:
                bad.append((type(inst).__name__, inst.name, str(inst.engine),
                            len(si.on_wait)))
    return bad


class _TailRunwayPatch:
    """Plant runway nops on SP right before Tile's tail drain so the drain's
    many queue waits can be redistributed by _legalize_waits."""

    def __enter__(self):
        self.orig = tile.TileContext._drain_and_barrier
        orig = self.orig

        def patched(tc_self, tick_clock, wait_clock):
            for _ in range(_TAIL_RUNWAY):
                tc_self.nc.sync.nop(nofuse=True)
            return orig(tc_self, tick_clock, wait_clock)

        tile.TileContext._drain_and_barrier = patched
        return self

    def __exit__(self, *a):
        tile.TileContext._drain_and_barrier = self.orig


# ---------------------------------------------------------------- builder

def _build_nc(debug=False):
    nc = bass.Bass(trn_type="TRN2")

    xq = nc.dram_tensor("xq", [128, 4, NKC, 512], F16, kind="ExternalInput")
    wqkv = nc.dram_tensor("wqkv", [128, 6, NKC, 128], F16, kind="ExternalInput")
    wcc = nc.dram_tensor("wcc", [128, QH, 32, 128], F16, kind="ExternalInput")
    cs = nc.dram_tensor("cs", [128, T], F16, kind="ExternalInput")
    sn = nc.dram_tensor("sn", [128, T], F16, kind="ExternalInput")
    pmt = nc.dram_tensor("pmt", [128, 128], F16, kind="ExternalInput")
    idt = nc.dram_tensor("idt", [128, 128], F16, kind="ExternalInput")
    tri = nc.dram_tensor("tri", [128, 128], F16, kind="ExternalInput")
    bias = nc.dram_tensor("bias", [128, 6], F32, kind="ExternalInput")
    otT = nc.dram_tensor("otT", [32, 128, 4, 512], F16, kind="ExternalOutput")
    dbg = None
    if debug:
        dbg = {
            "dbg_qk": nc.dram_tensor("dbg_qk", [128, QH + 1, T], F16, kind="ExternalOutput"),
            "dbg_va": nc.dram_tensor("dbg_va", [128, 16, 129], F16, kind="ExternalOutput"),
            "dbg_yt": nc.dram_tensor("dbg_yt", [128, QH, T], F16, kind="ExternalOutput"),
        }

    with _TailRunwayPatch(), tile.TileContext(nc) as tc:
        _trace_body(nc, tc, xq, wqkv, wcc, cs, sn, pmt, idt, tri, bias, otT, dbg)

    _legalize_waits(nc)
    bad = _audit(nc)
    if bad:
        raise RuntimeError(f"multi-wait instructions remain: {bad[:10]}")
    return nc


def _trace_body(nc, tc, xq, wqkv, wcc, cs, sn, pmt, idt, tri, bias, otT, dbg=None):
    persist = ExitStack()

    # ---------------- persistent SBUF ----------------
    misc = persist.enter_context(tc.tile_pool(name="misc", bufs=1))
    qk_pool = persist.enter_context(tc.tile_pool(name="qkp", bufs=1))
    va_pool = persist.enter_context(tc.tile_pool(name="vap", bufs=1))
    yt_pool = persist.enter_context(tc.tile_pool(name="ytp", bufs=1))

    cs_sb = misc.tile([128, T], F16)
    sn_sb = misc.tile([128, T], F16)
    pm_sb = misc.tile([128, 128], F16)
    idt_sb = misc.tile([128, 128], F16)
    tri_sb = misc.tile([128, 128], F16)
    bias_sb = misc.tile([128, 6], F32)

    qkT = qk_pool.tile([128, QH + 1, T], F16)      # q heads 0..3, k at 4
    v_aug = va_pool.tile([128, 16, 130], F16)      # [keys, kt, HS+ones]
    nc.vector.memset(v_aug[:, :, 128:129], 1.0)
    yT = yt_pool.tile([128, QH, T], F16)

    pa = persist.enter_context(tc.tile_pool(name="pa", bufs=2, space="PSUM"))

    # attention pools (outlive phase 1, so opened first — LIFO pool stack)
    ph3s = ExitStack()
    pt_pool = ph3s.enter_context(tc.tile_pool(name="pt", bufs=3))
    yn_pool = ph3s.enter_context(tc.tile_pool(name="yn", bufs=3))
    li_pool = ph3s.enter_context(tc.tile_pool(name="li", bufs=3))
    ph3p = ExitStack()
    ps = ph3p.enter_context(tc.tile_pool(name="ps", bufs=2, space="PSUM"))
    py = ph3p.enter_context(tc.tile_pool(name="py", bufs=4, space="PSUM"))

    # ---------------- phase 1 pools ----------------
    ph1 = ExitStack()
    w_pool = ph1.enter_context(tc.tile_pool(name="wqkv", bufs=1))
    xt_pool = ph1.enter_context(tc.tile_pool(name="xt", bufs=2))
    qraw_pool = ph1.enter_context(tc.tile_pool(name="qraw", bufs=2))
    ta_pool = ph1.enter_context(tc.tile_pool(name="ta", bufs=2))
    vt_pool = ph1.enter_context(tc.tile_pool(name="vt", bufs=2))

    wqkv_sb = w_pool.tile([128, 6, NKC, 128], F16)
    # chunk order: k, v first (unblocks attention), then q heads
    order = [KIDX, 5, 0, 1, 2, 3]
    # startup-critical DMA order: tiny constants, chunk-k weights, quarter-0
    # x in 8-kc slices (first proj matmul starts after ~2MB, not ~13MB),
    # then the remaining weights in consumption order
    nc.sync.dma_start(out=pm_sb, in_=pmt[:, :])
    nc.sync.dma_start(out=idt_sb, in_=idt[:, :])
    nc.sync.dma_start(out=tri_sb, in_=tri[:, :])
    nc.sync.dma_start(out=bias_sb, in_=bias[:, :])
    nc.sync.dma_start(out=wqkv_sb[:, KIDX, :, :], in_=wqkv[:, KIDX, :, :])
    xt0_sb = xt_pool.tile([128, NKC, 512], F16, tag="xt")
    for g in range(4):
        nc.sync.dma_start(out=xt0_sb[:, g * 8:(g + 1) * 8, :],
                          in_=xq[:, 0, g * 8:(g + 1) * 8, :])
    nc.sync.dma_start(out=wqkv_sb[:, 5, :, :], in_=wqkv[:, 5, :, :])
    nc.sync.dma_start(out=wqkv_sb[:, 0, :, :], in_=wqkv[:, 0, :, :])
    nc.sync.dma_start(out=cs_sb, in_=cs[:, :])
    nc.sync.dma_start(out=sn_sb, in_=sn[:, :])
    for c in (1, 2, 3):
        nc.sync.dma_start(out=wqkv_sb[:, c, :, :], in_=wqkv[:, c, :, :])

    xt_tiles = {0: xt0_sb}

    def prefetch_quarter(qt):
        xt_sb = xt_pool.tile([128, NKC, 512], F16, tag="xt")
        nc.sync.dma_start(out=xt_sb, in_=xq[:, qt, :, :])
        xt_tiles[qt] = xt_sb

    def proj_quarter(qt):
        tsl = slice(qt * 512, (qt + 1) * 512)
        xt_sb = xt_tiles.pop(qt)
        pending = None
        for c in order:
            acc = pa.tile([128, 512], F32, tag="pa")
            for kc in range(NKC):
                nc.tensor.matmul(acc, wqkv_sb[:, c, kc, :], xt_sb[:, kc, :],
                                 start=(kc == 0), stop=(kc == NKC - 1),
                                 skip_group_check=True)
            # previous chunk's PE epilogue lands after this chunk's matmuls
            # so its ACT-drain latency is hidden
            if pending is not None:
                _emit_pending(pending, qt, tsl)
            if c == 5:   # v: drain with bias, then transpose to natural
                vt_sb = vt_pool.tile([128, 512], F16)
                nc.scalar.activation(out=vt_sb, in_=acc, func=IDENT,
                                     bias=bias_sb[:, 4:5], scale=1.0)
                pending = ("v", vt_sb)
            else:        # q head c (or k): drain (+bias), RoPE
                qraw = qraw_pool.tile([128, 512], F16)
                if c == KIDX:
                    nc.scalar.copy(out=qraw, in_=acc)
                else:
                    nc.scalar.activation(out=qraw, in_=acc, func=IDENT,
                                         bias=bias_sb[:, c:c + 1], scale=1.0)
                pending = ("rope", c, qraw)
        _emit_pending(pending, qt, tsl)

    def _emit_pending(pending, qt, tsl):
        # PE work for the previous chunk, emitted after the next chunk's
        # accumulation matmuls so the ACT drain latency is hidden.
        if pending[0] == "v":
            vt_sb = pending[1]
            for i in range(4):
                kt = qt * 4 + i
                vtr = pa.tile([128, 128], F16, tag="pa")
                nc.tensor.matmul(vtr, vt_sb[:, i * 128:(i + 1) * 128], idt_sb,
                                 is_transpose=True, skip_group_check=True)
                nc.vector.tensor_copy(out=v_aug[:, kt, 0:128], in_=vtr)
        else:
            _, c, qraw = pending
            rot = pa.tile([128, 512], F32, tag="pa")
            nc.tensor.matmul(rot, pm_sb, qraw, start=True, stop=True,
                             skip_group_check=True)
            dst = qkT[:, c, tsl]
            ta = ta_pool.tile([128, 512], F16)
            nc.vector.tensor_mul(ta, rot, sn_sb[:, tsl])
            nc.vector.tensor_mul(dst, qraw, cs_sb[:, tsl])
            nc.vector.tensor_add(dst, dst, ta)

    def attn_group(qb, h):
        nkc = 4 * qb + 4
        # two 2-qsub accumulators: each [2,130] f32 region fits one PSUM bank
        y_lo = py.tile([128, 2, 130], F32, tag="y")
        y_hi = py.tile([128, 2, 130], F32, tag="y")
        y_of = lambda qsub: (y_lo if qsub < 2 else y_hi)[:, qsub % 2, :]
        pts = {}

        def s_and_exp(kc):
            qs0 = max(0, kc - 4 * qb)
            w = 512 - qs0 * 128
            s_ps = ps.tile([128, 512], F32, tag="ps")
            nc.tensor.matmul(s_ps[:, 0:w],
                             qkT[:, KIDX, kc * 128:(kc + 1) * 128],
                             qkT[:, h, qb * 512 + qs0 * 128:(qb + 1) * 512],
                             start=True, stop=True, skip_group_check=True)
            pt = pt_pool.tile([128, 512], F16)
            nc.scalar.activation(out=pt[:, 0:w], in_=s_ps[:, 0:w], func=EXP,
                                 bias=bias_sb[:, 5:6], scale=INV_SQRT_HS)
            if kc >= 4 * qb:
                j = kc - 4 * qb - qs0   # diagonal tile, pt-local index
                nc.vector.tensor_mul(pt[:, j * 128:(j + 1) * 128],
                                     pt[:, j * 128:(j + 1) * 128], tri_sb)
            pts[kc] = (pt, qs0)

        def y_mms(kc):
            pt, qs0 = pts.pop(kc)
            for qsub in range(qs0, 4):
                # start=True clears has_written for the WHOLE 2KB psum bank,
                # so only the first matmul into each [2,130] tile may set it;
                # the sibling region's first write zero-overwrites via the
                # pending-zero mechanism.
                nc.tensor.matmul(y_of(qsub)[:, 0:129],
                                 pt[:, (qsub - qs0) * 128:(qsub - qs0 + 1) * 128],
                                 v_aug[:, kc, 0:129],
                                 start=(kc == 0 and qsub % 2 == 0),
                                 stop=(kc == 4 * qb + qsub),
                                 skip_group_check=True)

        s_and_exp(0)
        if nkc > 1:
            s_and_exp(1)
        for kc in range(nkc):
            if kc + 2 < nkc:
                s_and_exp(kc + 2)
            y_mms(kc)

        for qsub in range(4):
            linv = li_pool.tile([128, 1], F32)
            nc.vector.reciprocal(linv, y_of(qsub)[:, 128:129])
            yn = yn_pool.tile([128, 128], F16)
            nc.vector.tensor_scalar_mul(yn, y_of(qsub)[:, 0:128], linv)
            ytr = ps.tile([128, 128], F16, tag="ps")
            nc.tensor.matmul(ytr, yn, idt_sb, is_transpose=True,
                             skip_group_check=True)
            nc.vector.tensor_copy(
                out=yT[:, h, (qb * 4 + qsub) * 128:(qb * 4 + qsub + 1) * 128],
                in_=ytr)

    # ---------------- phases 1-3 interleaved ----------------
    for qt in range(4):
        proj_quarter(qt)
        if qt < 3:
            prefetch_quarter(qt + 1)
        for h in range(QH):
            attn_group(qt, h)

    ph1.close()

    if dbg is not None:
        nc.sync.dma_start(out=dbg["dbg_qk"][:, :, :], in_=qkT)
        nc.sync.dma_start(out=dbg["dbg_va"][:, :, :], in_=v_aug[:, :, 0:129])
        nc.sync.dma_start(out=dbg["dbg_yt"][:, :, :], in_=yT)

    # ---------------- phase 4: c_proj -> out^T ----------------
    tail = ExitStack()
    wc_pool = tail.enter_context(tc.tile_pool(name="wc", bufs=1))
    wc_sb = wc_pool.tile([128, QH, 32, 128], F16)
    nc.sync.dma_start(out=wc_sb, in_=wcc[:, :, :, :])

    ph3p.close()

    ph4 = ExitStack()
    oc_pool = ph4.enter_context(tc.tile_pool(name="oc", bufs=3))
    pc = ph4.enter_context(tc.tile_pool(name="pc", bufs=2, space="PSUM"))

    for cb in range(32):
        oc_sb = oc_pool.tile([128, 4, 512], F16)
        for quad in range(4):
            occ = pc.tile([128, 512], F32)
            for hd in range(QH):
                nc.tensor.matmul(occ, wc_sb[:, hd, cb, :],
                                 yT[:, hd, quad * 512:(quad + 1) * 512],
                                 start=(hd == 0), stop=(hd == QH - 1),
                                 skip_group_check=True)
            nc.scalar.copy(out=oc_sb[:, quad, :], in_=occ)
        nc.sync.dma_start(out=otT[cb, :, :, :], in_=oc_sb)

    ph4.close()
    tail.close()
    ph3s.close()
    persist.close()


# ---------------------------------------------------------------- host side

def _rope_T_np(seq_len, hs):
    inv_freq = 1.0 / (SCALE * BASE ** (np.arange(0, hs, 2, dtype=np.float64) / hs))
    freqs = np.outer(inv_freq, np.arange(seq_len, dtype=np.float64))  # [64, T]
    emb = np.concatenate([freqs, freqs], axis=0)                      # [128, T]
    return np.cos(emb).astype(np.float16), np.sin(emb).astype(np.float16)


_CACHE = {}


def _get_nc():
    if "nc" not in _CACHE:
        _CACHE["nc"] = _build_nc()
    return _CACHE["nc"]


def kernel(q_x, Wq, bq, Wk, bk, Wv, bv, Wc, bc, _trace=False):
    q_x = np.asarray(q_x, dtype=np.float32)
    Wq = np.asarray(Wq, dtype=np.float32)
    Wk = np.asarray(Wk, dtype=np.float32)
    Wv = np.asarray(Wv, dtype=np.float32)
    Wc = np.asarray(Wc, dtype=np.float32)
    bq = np.asarray(bq, dtype=np.float32)
    bv = np.asarray(bv, dtype=np.float32)
    bc = np.asarray(bc, dtype=np.float32)
    # NOTE: bk is exactly softmax-invariant (adds a per-query constant to all
    # scores) so it is dropped on device.

    x = q_x.reshape(T, C)
    xT = np.ascontiguousarray(x.T).astype(np.float16)          # [C, T]
    # xq[p, qt, kc, j] = xT[kc*128+p, qt*512+j]
    xq = np.ascontiguousarray(
        xT.reshape(NKC, 128, 4, 512).transpose(1, 2, 0, 3))

    cosT, snT = _rope_T_np(T, HS)

    pm = np.zeros((128, 128), np.float16)
    for m in range(64):
        pm[m + 64, m] = -1.0
        pm[m, m + 64] = 1.0
    ident = np.eye(128, dtype=np.float16)
    tri = (np.arange(128)[:, None] <= np.arange(128)[None, :]).astype(np.float16)

    in_maps = []
    for c in range(NCORES):
        wq_c = Wq[c * DQ:(c + 1) * DQ, :]                # [512, C]
        wk_c = Wk[c * HS:(c + 1) * HS, :]                # [128, C]
        wv_c = Wv[c * HS:(c + 1) * HS, :]
        wcat = np.concatenate([wq_c, wk_c, wv_c], axis=0).astype(np.float16)
        # wqkv[p, ch, kc, j] = wcat[ch*128+j, kc*128+p]
        wqkv_a = np.ascontiguousarray(
            wcat.reshape(6, 128, NKC, 128).transpose(3, 0, 2, 1))
        wcT = np.ascontiguousarray(Wc[:, c * DQ:(c + 1) * DQ].T).astype(np.float16)
        # wcc[p, hd, cb, j] = wcT[hd*128+p, cb*128+j]
        wcc_a = np.ascontiguousarray(
            wcT.reshape(QH, 128, 32, 128).transpose(1, 0, 2, 3))
        bias_a = np.zeros((128, 6), np.float32)
        for h in range(QH):
            bias_a[:, h] = bq[c * DQ + h * HS: c * DQ + (h + 1) * HS]
        bias_a[:, 4] = bv[c * HS:(c + 1) * HS]
        bias_a[:, 5] = -EXP_SHIFT
        in_maps.append({
            "xq": xq, "wqkv": wqkv_a, "wcc": wcc_a, "cs": cosT, "sn": snT,
            "pmt": pm, "idt": ident, "tri": tri, "bias": bias_a,
        })

    nc = _get_nc()
    res = run_bass_kernel_spmd(nc, in_maps, core_ids=list(range(NCORES)),
                               trace=_trace)
    acc = np.zeros((C, T), dtype=np.float64)
    for c in range(NCORES):
        acc += res.results[c]["otT"].reshape(C, T).astype(np.float64)
    out = (acc.T + bc.astype(np.float64)[None, :]).astype(np.float32)
    if _trace:
        _CACHE["last_exec_time_ns"] = res.exec_time_ns
        _CACHE["last_results"] = res
    return out.reshape(B, T, C)


# revision 37
# speedup vs baseline: 2.2384x; 1.0616x over previous
"""Trainium2 Bass kernel for a GQA attention block (B=1, T=2048, C=4096,
NH=32, NKV=8, HS=128), tensor-parallel over heads across 8 NeuronCores.

Per core c: 4 query heads (4c..4c+3) and 1 KV head (c). Everything on the PE
path is fp16 (same PE throughput as fp32r, half the LDWEIGHTS time, half the
DMA bytes, 2x DVE modes, ~16x less rounding than bf16):

  - projections W-stationary: out = W^T-chunk stationary, x^T moving ->
    q^T/k^T/v^T [HS, T] directly (no per-tile PE transposes); bias fused
    into the ACT PSUM->SBUF drain.
  - RoPE rotate-half via a constant +-1 permutation matmul on PE (handles
    the cross-partition shuffle), then 3 DVE ops (mul/mul/add) per chunk.
  - attention: S^T = k-chunk^T q [keys, queries] -> ACT exp(s/sqrt(d) - 8)
    (shift keeps fp16 in range; cancels in normalization) -> diagonal-tile
    triangle mask multiply -> y natural [128q, 129] with a ones-column
    appended to V so the softmax denominator accumulates for free ->
    per-partition reciprocal [128,1] + scale -> PE transpose to y^T.
    Causal trim: no upper-triangle tiles are computed.
  - c_proj W-stationary producing out^T [C, T] fp16 partials (host
    transposes and sums across cores = the TP all-reduce).
  - ~50 batched DMA instructions total (host pre-arranges every operand so
    each DMA is a [128, contiguous-bytes] blit).
"""
import sys

sys.path.insert(0, "/opt/trn_rl_repo")

import numpy as np

from contextlib import ExitStack

import concourse.bass as bass
import concourse.mybir as mybir
import concourse.tile as tile
from concourse.bass_utils import run_bass_kernel_spmd

# ---------------------------------------------------------------- constants
B, T, C = 1, 2048, 4096
NH, NKV, HS = 32, 8, 128
NCORES = 8
QH = NH // NCORES          # 4 query heads per core
DQ = QH * HS               # 512
NKC = C // 128             # 32 contraction chunks
BASE, SCALE = 10000.0, 1.0
INV_SQRT_HS = 1.0 / float(np.sqrt(HS))
EXP_SHIFT = 8.0
KIDX = 4                   # k's surface index in qkT / wqkv chunk order

F32 = mybir.dt.float32
F16 = mybir.dt.float16
IDENT = mybir.ActivationFunctionType.Identity
EXP = mybir.ActivationFunctionType.Exp

# ------------------------------------------------------- wait legalization
_TAIL_RUNWAY = 48


def _legalize_waits(nc):
    """walrus (this toolchain) allows ONE sync wait per ISA instruction.
    Split excess waits off onto standalone EventSemaphore instructions
    inserted immediately before the offender (same engine stream order)."""
    n_split = 0
    for bb in nc.m.functions[0].blocks:
        insts = bb.instructions
        if not any(i.sync_info and i.sync_info.on_wait and
                   len(i.sync_info.on_wait) > (0 if type(i).__name__ == "InstISA" else 1)
                   for i in insts):
            continue
        new_list = []
        for inst in insts:
            si = inst.sync_info
            is_raw_isa = type(inst).__name__ == "InstISA"
            keep_n = 0 if is_raw_isa else 1
            if si and si.on_wait and len(si.on_wait) > keep_n:
                waits = list(si.on_wait)
                split_off = waits if is_raw_isa else waits[:-1]
                for w in split_off:
                    ev = mybir.InstNoOp(
                        name=f"legal-wait-{nc.next_id()}",
                        ins=[], outs=[], engine=inst.engine,
                        bass_nofuse=True,
                        sync_info=mybir.SyncInfo(on_wait=[w], on_update=[]))
                    nc.register_instruction(ev, overwrite=True)
                    new_list.append(ev)
                    n_split += 1
                inst.sync_info = mybir.SyncInfo(
                    on_wait=[] if is_raw_isa else [waits[-1]],
                    on_update=list(si.on_update))
            new_list.append(inst)
        bb.instructions = new_list
    return n_split


def _audit(nc):
    bad = []
    for bb in nc.m.functions[0].blocks:
        for inst in bb.instructions:
            si = inst.sync_info
            if si and si.on_wait and len(si.on_wait) > 1:
                bad.append((type(inst).__name__, inst.name, str(inst.engine),
                            len(si.on_wait)))
    return bad


class _TailRunwayPatch:
    """Plant runway nops on SP right before Tile's tail drain so the drain's
    many queue waits can be redistributed by _legalize_waits."""

    def __enter__(self):
        self.orig = tile.TileContext._drain_and_barrier
        orig = self.orig

        def patched(tc_self, tick_clock, wait_clock):
            for _ in range(_TAIL_RUNWAY):
                tc_self.nc.sync.nop(nofuse=True)
            return orig(tc_self, tick_clock, wait_clock)

        tile.TileContext._drain_and_barrier = patched
        return self

    def __exit__(self, *a):
        tile.TileContext._drain_and_barrier = self.orig


# ---------------------------------------------------------------- builder

def _build_nc(debug=False):
    nc = bass.Bass(trn_type="TRN2")

    xq = nc.dram_tensor("xq", [128, 4, NKC, 512], F16, kind="ExternalInput")
    wqkv = nc.dram_tensor("wqkv", [128, 6, NKC, 128], F16, kind="ExternalInput")
    wcc = nc.dram_tensor("wcc", [128, QH, 32, 128], F16, kind="ExternalInput")
    cs = nc.dram_tensor("cs", [128, T], F16, kind="ExternalInput")
    sn = nc.dram_tensor("sn", [128, T], F16, kind="ExternalInput")
    pmt = nc.dram_tensor("pmt", [128, 128], F16, kind="ExternalInput")
    idt = nc.dram_tensor("idt", [128, 128], F16, kind="ExternalInput")
    tri = nc.dram_tensor("tri", [128, 128], F16, kind="ExternalInput")
    bias = nc.dram_tensor("bias", [128, 6], F32, kind="ExternalInput")
    otT = nc.dram_tensor("otT", [32, 128, 4, 512], F16, kind="ExternalOutput")
    dbg = None
    if debug:
        dbg = {
            "dbg_qk": nc.dram_tensor("dbg_qk", [128, QH + 1, T], F16, kind="ExternalOutput"),
            "dbg_va": nc.dram_tensor("dbg_va", [128, 16, 129], F16, kind="ExternalOutput"),
            "dbg_yt": nc.dram_tensor("dbg_yt", [128, QH, T], F16, kind="ExternalOutput"),
        }

    with _TailRunwayPatch(), tile.TileContext(nc) as tc:
        _trace_body(nc, tc, xq, wqkv, wcc, cs, sn, pmt, idt, tri, bias, otT, dbg)

    _legalize_waits(nc)
    bad = _audit(nc)
    if bad:
        raise RuntimeError(f"multi-wait instructions remain: {bad[:10]}")
    return nc


def _trace_body(nc, tc, xq, wqkv, wcc, cs, sn, pmt, idt, tri, bias, otT, dbg=None):
    persist = ExitStack()

    # ---------------- persistent SBUF ----------------
    misc = persist.enter_context(tc.tile_pool(name="misc", bufs=1))
    qk_pool = persist.enter_context(tc.tile_pool(name="qkp", bufs=1))
    va_pool = persist.enter_context(tc.tile_pool(name="vap", bufs=1))
    yt_pool = persist.enter_context(tc.tile_pool(name="ytp", bufs=1))

    cs_sb = misc.tile([128, T], F16)
    sn_sb = misc.tile([128, T], F16)
    pm_sb = misc.tile([128, 128], F16)
    idt_sb = misc.tile([128, 128], F16)
    tri_sb = misc.tile([128, 128], F16)
    bias_sb = misc.tile([128, 6], F32)

    qkT = qk_pool.tile([128, QH + 1, T], F16)      # q heads 0..3, k at 4
    v_aug = va_pool.tile([128, 16, 130], F16)      # [keys, kt, HS+ones]
    nc.vector.memset(v_aug[:, :, 128:129], 1.0)
    yT = yt_pool.tile([128, QH, T], F16)

    # attention pools (outlive phase 1, so opened first — LIFO pool stack)
    ph3s = ExitStack()
    pt_pool = ph3s.enter_context(tc.tile_pool(name="pt", bufs=3))
    yn_pool = ph3s.enter_context(tc.tile_pool(name="yn", bufs=3))
    li_pool = ph3s.enter_context(tc.tile_pool(name="li", bufs=3))
    ph3p = ExitStack()
    ps = ph3p.enter_context(tc.tile_pool(name="ps", bufs=2, space="PSUM"))
    py = ph3p.enter_context(tc.tile_pool(name="py", bufs=4, space="PSUM"))

    # ---------------- phase 1 pools ----------------
    ph1 = ExitStack()
    w_pool = ph1.enter_context(tc.tile_pool(name="wqkv", bufs=1))
    xt_pool = ph1.enter_context(tc.tile_pool(name="xt", bufs=2))
    qraw_pool = ph1.enter_context(tc.tile_pool(name="qraw", bufs=2))
    ta_pool = ph1.enter_context(tc.tile_pool(name="ta", bufs=2))
    vt_pool = ph1.enter_context(tc.tile_pool(name="vt", bufs=2))
    pa = ph1.enter_context(tc.tile_pool(name="pa", bufs=2, space="PSUM"))

    wqkv_sb = w_pool.tile([128, 6, NKC, 128], F16)
    # chunk order: k, v first (unblocks attention), then q heads
    order = [KIDX, 5, 0, 1, 2, 3]
    # startup-critical DMA order: tiny constants, chunk-k weights, quarter-0
    # x in 8-kc slices (first proj matmul starts after ~2MB, not ~13MB),
    # then the remaining weights in consumption order
    nc.sync.dma_start(out=pm_sb, in_=pmt[:, :])
    nc.sync.dma_start(out=idt_sb, in_=idt[:, :])
    nc.sync.dma_start(out=tri_sb, in_=tri[:, :])
    nc.sync.dma_start(out=bias_sb, in_=bias[:, :])
    nc.sync.dma_start(out=wqkv_sb[:, KIDX, :, :], in_=wqkv[:, KIDX, :, :])
    xt0_sb = xt_pool.tile([128, NKC, 512], F16, tag="xt")
    for g in range(4):
        nc.sync.dma_start(out=xt0_sb[:, g * 8:(g + 1) * 8, :],
                          in_=xq[:, 0, g * 8:(g + 1) * 8, :])
    nc.sync.dma_start(out=wqkv_sb[:, 5, :, :], in_=wqkv[:, 5, :, :])
    nc.sync.dma_start(out=wqkv_sb[:, 0, :, :], in_=wqkv[:, 0, :, :])
    nc.sync.dma_start(out=cs_sb, in_=cs[:, :])
    nc.sync.dma_start(out=sn_sb, in_=sn[:, :])
    for c in (1, 2, 3):
        nc.sync.dma_start(out=wqkv_sb[:, c, :, :], in_=wqkv[:, c, :, :])

    xt_tiles = {0: xt0_sb}

    def prefetch_quarter(qt):
        xt_sb = xt_pool.tile([128, NKC, 512], F16, tag="xt")
        nc.sync.dma_start(out=xt_sb, in_=xq[:, qt, :, :])
        xt_tiles[qt] = xt_sb

    def quarter_gen(qt):
        """Generator emitting quarter qt's projection work in small PE units.
        Driven as filler between attention kc-steps (which are ACT/exp-bound)
        so the PE never idles waiting on the scalar engine."""
        tsl = slice(qt * 512, (qt + 1) * 512)
        xt_sb = xt_tiles.pop(qt)
        pending = None
        for c in order:
            acc = pa.tile([128, 512], F32, tag="pa")
            for kc in range(NKC):
                nc.tensor.matmul(acc, wqkv_sb[:, c, kc, :], xt_sb[:, kc, :],
                                 start=(kc == 0), stop=(kc == NKC - 1),
                                 skip_group_check=True)
                if kc % 4 == 3:
                    yield
            # previous chunk's PE epilogue lands after this chunk's matmuls
            # so its ACT-drain latency is hidden
            if pending is not None:
                _emit_pending(pending, qt, tsl)
                yield
            if c == 5:   # v: drain with bias, then transpose to natural
                vt_sb = vt_pool.tile([128, 512], F16)
                nc.scalar.activation(out=vt_sb, in_=acc, func=IDENT,
                                     bias=bias_sb[:, 4:5], scale=1.0)
                pending = ("v", vt_sb)
            else:        # q head c (or k): drain (+bias), RoPE
                qraw = qraw_pool.tile([128, 512], F16)
                if c == KIDX:
                    nc.scalar.copy(out=qraw, in_=acc)
                else:
                    nc.scalar.activation(out=qraw, in_=acc, func=IDENT,
                                         bias=bias_sb[:, c:c + 1], scale=1.0)
                pending = ("rope", c, qraw)
        _emit_pending(pending, qt, tsl)

    def _emit_pending(pending, qt, tsl):
        # PE work for the previous chunk, emitted after the next chunk's
        # accumulation matmuls so the ACT drain latency is hidden.
        if pending[0] == "v":
            vt_sb = pending[1]
            for i in range(4):
                kt = qt * 4 + i
                vtr = pa.tile([128, 128], F16, tag="pa")
                nc.tensor.matmul(vtr, vt_sb[:, i * 128:(i + 1) * 128], idt_sb,
                                 is_transpose=True, skip_group_check=True)
                nc.vector.tensor_copy(out=v_aug[:, kt, 0:128], in_=vtr)
        else:
            _, c, qraw = pending
            rot = pa.tile([128, 512], F32, tag="pa")
            nc.tensor.matmul(rot, pm_sb, qraw, start=True, stop=True,
                             skip_group_check=True)
            dst = qkT[:, c, tsl]
            ta = ta_pool.tile([128, 512], F16)
            nc.vector.tensor_mul(ta, rot, sn_sb[:, tsl])
            nc.vector.tensor_mul(dst, qraw, cs_sb[:, tsl])
            nc.vector.tensor_add(dst, dst, ta)

    def attn_group(qb, h, filler=None, fill_n=1):
        nkc = 4 * qb + 4
        # two 2-qsub accumulators: each [2,130] f32 region fits one PSUM bank
        y_lo = py.tile([128, 2, 130], F32, tag="y")
        y_hi = py.tile([128, 2, 130], F32, tag="y")
        y_of = lambda qsub: (y_lo if qsub < 2 else y_hi)[:, qsub % 2, :]
        pts = {}

        def s_and_exp(kc):
            qs0 = max(0, kc - 4 * qb)
            w = 512 - qs0 * 128
            s_ps = ps.tile([128, 512], F32, tag="ps")
            nc.tensor.matmul(s_ps[:, 0:w],
                             qkT[:, KIDX, kc * 128:(kc + 1) * 128],
                             qkT[:, h, qb * 512 + qs0 * 128:(qb + 1) * 512],
                             start=True, stop=True, skip_group_check=True)
            pt = pt_pool.tile([128, 512], F16)
            nc.scalar.activation(out=pt[:, 0:w], in_=s_ps[:, 0:w], func=EXP,
                                 bias=bias_sb[:, 5:6], scale=INV_SQRT_HS)
            if kc >= 4 * qb:
                j = kc - 4 * qb - qs0   # diagonal tile, pt-local index
                nc.vector.tensor_mul(pt[:, j * 128:(j + 1) * 128],
                                     pt[:, j * 128:(j + 1) * 128], tri_sb)
            pts[kc] = (pt, qs0)

        def y_mms(kc):
            pt, qs0 = pts.pop(kc)
            for qsub in range(qs0, 4):
                # start=True clears has_written for the WHOLE 2KB psum bank,
                # so only the first matmul into each [2,130] tile may set it;
                # the sibling region's first write zero-overwrites via the
                # pending-zero mechanism.
                nc.tensor.matmul(y_of(qsub)[:, 0:129],
                                 pt[:, (qsub - qs0) * 128:(qsub - qs0 + 1) * 128],
                                 v_aug[:, kc, 0:129],
                                 start=(kc == 0 and qsub % 2 == 0),
                                 stop=(kc == 4 * qb + qsub),
                                 skip_group_check=True)

        def fill():
            if filler is not None:
                for _ in range(fill_n):
                    if next(filler, None) is None:
                        break

        s_and_exp(0)
        if nkc > 1:
            s_and_exp(1)
        for kc in range(nkc):
            if kc + 2 < nkc:
                s_and_exp(kc + 2)
            fill()
            y_mms(kc)

        for qsub in range(4):
            linv = li_pool.tile([128, 1], F32)
            nc.vector.reciprocal(linv, y_of(qsub)[:, 128:129])
            yn = yn_pool.tile([128, 128], F16)
            nc.vector.tensor_scalar_mul(yn, y_of(qsub)[:, 0:128], linv)
            ytr = ps.tile([128, 128], F16, tag="ps")
            nc.tensor.matmul(ytr, yn, idt_sb, is_transpose=True,
                             skip_group_check=True)
            nc.vector.tensor_copy(
                out=yT[:, h, (qb * 4 + qsub) * 128:(qb * 4 + qsub + 1) * 128],
                in_=ytr)

    # ------- phases 1-3 software-pipelined: quarter qt's projections are
    # dribbled into quarter qt-1's (ACT-bound) attention as PE filler -------
    for _ in quarter_gen(0):
        pass
    prefetch_quarter(1)
    FILL_N = {0: 4, 1: 2, 2: 1}
    for qt in (1, 2, 3):
        g = quarter_gen(qt)
        for h in range(QH):
            attn_group(qt - 1, h, filler=g, fill_n=FILL_N[qt - 1])
        for _ in g:
            pass
        if qt < 3:
            prefetch_quarter(qt + 1)

    ph1.close()

    if dbg is not None:
        nc.sync.dma_start(out=dbg["dbg_qk"][:, :, :], in_=qkT)
        nc.sync.dma_start(out=dbg["dbg_va"][:, :, :], in_=v_aug[:, :, 0:129])

    # ---------------- phase 4: c_proj -> out^T ----------------
    # quads 0-2 (t < 1536) depend only on attention qb<=2, so they fill the
    # final ACT-bound attention block qb=3; quad 3 is the tail.
    tail = ExitStack()
    wc_pool = tail.enter_context(tc.tile_pool(name="wc", bufs=1))
    wc_sb = wc_pool.tile([128, QH, 32, 128], F16)
    nc.sync.dma_start(out=wc_sb, in_=wcc[:, :, :, :])

    ph4 = ExitStack()
    oc_pool = ph4.enter_context(tc.tile_pool(name="oc", bufs=4))
    pc = ph4.enter_context(tc.tile_pool(name="pc", bufs=2, space="PSUM"))

    def cproj_quads(quads):
        for quad in quads:
            for cb in range(32):
                occ = pc.tile([128, 512], F32)
                for hd in range(QH):
                    nc.tensor.matmul(occ, wc_sb[:, hd, cb, :],
                                     yT[:, hd, quad * 512:(quad + 1) * 512],
                                     start=(hd == 0), stop=(hd == QH - 1),
                                     skip_group_check=True)
                oc_sb = oc_pool.tile([128, 512], F16)
                nc.scalar.copy(out=oc_sb, in_=occ)
                nc.sync.dma_start(out=otT[cb, :, quad, :], in_=oc_sb)
                yield

    # h=0 group runs unfilled (covers the wc DMA latency), then quads 0-2
    # dribble into groups h=1..3
    cg = cproj_quads((0, 1, 2))
    attn_group(3, 0)
    for h in (1, 2, 3):
        attn_group(3, h, filler=cg, fill_n=2)
    for _ in cg:
        pass
    for _ in cproj_quads((3,)):
        pass

    if dbg is not None:
        nc.sync.dma_start(out=dbg["dbg_yt"][:, :, :], in_=yT)

    ph4.close()
    tail.close()
    ph3p.close()
    ph3s.close()
    persist.close()


# ---------------------------------------------------------------- host side

def _rope_T_np(seq_len, hs):
    inv_freq = 1.0 / (SCALE * BASE ** (np.arange(0, hs, 2, dtype=np.float64) / hs))
    freqs = np.outer(inv_freq, np.arange(seq_len, dtype=np.float64))  # [64, T]
    emb = np.concatenate([freqs, freqs], axis=0)                      # [128, T]
    return np.cos(emb).astype(np.float16), np.sin(emb).astype(np.float16)


_CACHE = {}


def _get_nc():
    if "nc" not in _CACHE:
        _CACHE["nc"] = _build_nc()
    return _CACHE["nc"]


def kernel(q_x, Wq, bq, Wk, bk, Wv, bv, Wc, bc, _trace=False):
    q_x = np.asarray(q_x, dtype=np.float32)
    Wq = np.asarray(Wq, dtype=np.float32)
    Wk = np.asarray(Wk, dtype=np.float32)
    Wv = np.asarray(Wv, dtype=np.float32)
    Wc = np.asarray(Wc, dtype=np.float32)
    bq = np.asarray(bq, dtype=np.float32)
    bv = np.asarray(bv, dtype=np.float32)
    bc = np.asarray(bc, dtype=np.float32)
    # NOTE: bk is exactly softmax-invariant (adds a per-query constant to all
    # scores) so it is dropped on device.

    x = q_x.reshape(T, C)
    xT = np.ascontiguousarray(x.T).astype(np.float16)          # [C, T]
    # xq[p, qt, kc, j] = xT[kc*128+p, qt*512+j]
    xq = np.ascontiguousarray(
        xT.reshape(NKC, 128, 4, 512).transpose(1, 2, 0, 3))

    cosT, snT = _rope_T_np(T, HS)

    pm = np.zeros((128, 128), np.float16)
    for m in range(64):
        pm[m + 64, m] = -1.0
        pm[m, m + 64] = 1.0
    ident = np.eye(128, dtype=np.float16)
    tri = (np.arange(128)[:, None] <= np.arange(128)[None, :]).astype(np.float16)

    in_maps = []
    for c in range(NCORES):
        wq_c = Wq[c * DQ:(c + 1) * DQ, :]                # [512, C]
        wk_c = Wk[c * HS:(c + 1) * HS, :]                # [128, C]
        wv_c = Wv[c * HS:(c + 1) * HS, :]
        wcat = np.concatenate([wq_c, wk_c, wv_c], axis=0).astype(np.float16)
        # wqkv[p, ch, kc, j] = wcat[ch*128+j, kc*128+p]
        wqkv_a = np.ascontiguousarray(
            wcat.reshape(6, 128, NKC, 128).transpose(3, 0, 2, 1))
        wcT = np.ascontiguousarray(Wc[:, c * DQ:(c + 1) * DQ].T).astype(np.float16)
        # wcc[p, hd, cb, j] = wcT[hd*128+p, cb*128+j]
        wcc_a = np.ascontiguousarray(
            wcT.reshape(QH, 128, 32, 128).transpose(1, 0, 2, 3))
        bias_a = np.zeros((128, 6), np.float32)
        for h in range(QH):
            bias_a[:, h] = bq[c * DQ + h * HS: c * DQ + (h + 1) * HS]
        bias_a[:, 4] = bv[c * HS:(c + 1) * HS]
        bias_a[:, 5] = -EXP_SHIFT
        in_maps.append({
            "xq": xq, "wqkv": wqkv_a, "wcc": wcc_a, "cs": cosT, "sn": snT,
            "pmt": pm, "idt": ident, "tri": tri, "bias": bias_a,
        })

    nc = _get_nc()
    res = run_bass_kernel_spmd(nc, in_maps, core_ids=list(range(NCORES)),
                               trace=_trace)
    acc = np.zeros((C, T), dtype=np.float64)
    for c in range(NCORES):
        acc += res.results[c]["otT"].reshape(C, T).astype(np.float64)
    out = (acc.T + bc.astype(np.float64)[None, :]).astype(np.float32)
    if _trace:
        _CACHE["last_exec_time_ns"] = res.exec_time_ns
        _CACHE["last_results"] = res
    return out.reshape(B, T, C)
